# revision 9
# baseline (speedup 1.0000x reference)
# Trainium2 Bass kernel for LinearAttention — v2 (q-path folded).
#
# Reference computation (per batch element b of 16):
#   qkv = w_qkv @ x[b]; q,k,v split into 8 heads x 64 dims
#   E = exp(k); ctx_h = (E_h/rowsum) @ v_h^T        # [64, 64]
#   y = w_out @ concat(ctx_h^T @ q_h) + b_out
#
# Key algebra: y = Wy @ x + b where
#   Wy = w_out @ blockdiag(ctx~_h^T) @ w_q   (per batch, [512, 512])
# so q never needs to be computed over l. Per batch:
#   Pass A: kT/vT projections (l on partitions), E = exp(kT), ctx
#           accumulation via head-pair matmuls with ones columns in vT
#           producing rowsums.
#   Fold:   ctx~ = ctx/rowsum; tmp_h = ctx~_h^T @ w_q_h;
#           WyT[c, y] = sum_h tmp_h^T-contract-w_outT  ([512, 512])
#   Pass Y: y = WyT^T-contract-x + bias; DMA out (fp16).
#
# All matmuls in fp16 (1 cycle/row at any N; fp32 PSUM accumulate).
# Data-parallel over batch: 16 batches / 8 cores = 2 per core.

import numpy as np
from contextlib import ExitStack

import concourse.bass as bass
import concourse.bacc as bacc
import concourse.mybir as mybir
import concourse.tile as tile

B, DIM, HGT, WID = 16, 512, 64, 64
L = HGT * WID            # 4096
HEADS, DH = 8, 64
HIDDEN = HEADS * DH      # 512
NCORES = 8
BPC = B // NCORES        # 2 batches per core
P = 128
CHUNK = 512
NCHUNK = L // CHUNK      # 8
KT = DIM // P            # 4 contraction tiles over channels
MT = DIM // P            # 4 output row tiles
LM = CHUNK // P          # 4 l-subtiles per chunk
NPAIR = HEADS // 2       # 4 head pairs
VW = DH + 2              # per-head vT width: 64 v cols + 2 ones cols

F32 = mybir.dt.float32
F16 = mybir.dt.float16


def build_kernel(ctx: ExitStack, tc: "tile.TileContext", x_in, wkvT_in, wq_in,
                 woT_in, bias_in, y_out):
    nc = tc.nc

    wpool = ctx.enter_context(tc.tile_pool(name="weights", bufs=1))
    xpool = ctx.enter_context(tc.tile_pool(name="xres", bufs=1))
    epool = ctx.enter_context(tc.tile_pool(name="ev", bufs=6))
    ypool = ctx.enter_context(tc.tile_pool(name="ysb", bufs=1))
    cpool = ctx.enter_context(tc.tile_pool(name="ctxacc", bufs=1))
    fpool = ctx.enter_context(tc.tile_pool(name="fold", bufs=1))
    wypool = ctx.enter_context(tc.tile_pool(name="wyt", bufs=1))
    psk = ctx.enter_context(tc.tile_pool(name="psk", bufs=2, space="PSUM"))
    psv = ctx.enter_context(tc.tile_pool(name="psv", bufs=2, space="PSUM"))
    psy = ctx.enter_context(tc.tile_pool(name="psy", bufs=2, space="PSUM"))
    psc = ctx.enter_context(tc.tile_pool(name="psc", bufs=2, space="PSUM"))

    # ---- k/v weights first (needed by the first matmuls), split into
    # k-half and v-half tiles so the first k matmuls wait on only 512KB ----
    dma_eng0 = [nc.sync, nc.scalar, nc.gpsimd]
    wk_sb, wv_sb = [], []
    for k in range(KT):
        t = wpool.tile([P, HIDDEN], F16, tag=f"wk{k}", name=f"wk{k}")
        dma_eng0[k % 3].dma_start(t[:], wkvT_in[k * P:(k + 1) * P, 0:HIDDEN])
        wk_sb.append(t)
    for k in range(KT):
        t = wpool.tile([P, HIDDEN], F16, tag=f"wv{k}", name=f"wv{k}")
        dma_eng0[(k + 1) % 3].dma_start(
            t[:], wkvT_in[k * P:(k + 1) * P, HIDDEN:2 * HIDDEN])
        wv_sb.append(t)

    # ---- resident x: chunk-pair tiles [128, 1024] (2KB rows for DMA
    # efficiency, 256KB dependency granularity for fast start), issued
    # pair-major round-robin over the three DMA-capable engine queues ----
    dma_engines = [nc.sync, nc.scalar, nc.gpsimd]
    PAIRW = 2 * CHUNK
    xj_sb = {}  # (b, kt, j) -> [128, 1024] fp16, j = chunk pair
    for b in range(BPC):
        for j in range(NCHUNK // 2):
            for k in range(KT):
                t = xpool.tile([P, PAIRW], F16, tag=f"x{b}_{k}_{j}",
                               name=f"x{b}_{k}_{j}")
                dma_engines[(k + j) % 3].dma_start(
                    t[:], x_in[b, k * P:(k + 1) * P,
                               j * PAIRW:(j + 1) * PAIRW])
                xj_sb[(b, k, j)] = t

    def x_sb(b, k, i):
        return xj_sb[(b, k, i // 2)][:, (i % 2) * CHUNK:(i % 2 + 1) * CHUNK]

    # fold-time weights (not needed until after pass A)
    wq_sb = []    # [128 hid-d, 512 c] per pair tile (wq rows as-is)
    for p in range(NPAIR):
        t = wpool.tile([P, DIM], F16, tag=f"wq{p}", name=f"wq{p}")
        nc.sync.dma_start(t[:], wq_in[p * P:(p + 1) * P, :])
        wq_sb.append(t)
    woT_sb = []   # [128 hid-e, 512 y] per pair tile (w_out^T rows)
    for p in range(NPAIR):
        t = wpool.tile([P, DIM], F16, tag=f"wo{p}", name=f"wo{p}")
        nc.scalar.dma_start(t[:], woT_in[p * P:(p + 1) * P, :])
        woT_sb.append(t)
    bias_sb = wpool.tile([P, MT], F32, tag="bias", name="bias")
    nc.gpsimd.dma_start(bias_sb[:], bias_in[:])

    wyT_sb = {}  # (b, m) -> [128 c-sub, 512 y] fp16

    def pass_A(b, y_batch=None, trailing_hook=None, ctx_acc=None):
        """kT/vT projections, exp, ctx accumulation for batch b; optionally
        interleaves the y pass of a previous batch. ctx groups of chunk i-1
        are emitted between the kv/y groups of chunk i so their small-N
        matmuls' LDWEIGHTS hide under big matmuls (queue reorder window).
        trailing_hook(p) is called right after pair p's last ctx group so
        per-pair finalize work overlaps the remaining ctx groups."""
        def ctx_group(i, p, E_t, vT_t):
            pc = psc.tile([P, 2 * VW], F32, tag="ctx", name="ctx")
            for lm in range(LM):
                nc.tensor.matmul(
                    pc[:], E_t[lm][:, p * P:(p + 1) * P],
                    vT_t[lm][:, p * 2 * VW:(p + 1) * 2 * VW],
                    start=(lm == 0), stop=(lm == LM - 1))
            if i == 0:
                nc.vector.tensor_copy(ctx_acc[p][:], pc[:])
            else:
                nc.vector.tensor_add(ctx_acc[p][:], ctx_acc[p][:], pc[:])

        prev = None  # (i-1, E_t, vT_t)
        for i in range(NCHUNK):
            E_t, vT_t = [], []
            for lm in range(LM):
                lsl = slice(lm * P, (lm + 1) * P)
                kps = psk.tile([P, CHUNK], F32, tag="kps", name="kps")
                for k in range(KT):
                    nc.tensor.matmul(kps[:], x_sb(b, k, i)[:, lsl],
                                     wk_sb[k][:],
                                     start=(k == 0), stop=(k == KT - 1))
                vps = psv.tile([P, CHUNK], F32, tag="vps", name="vps")
                for k in range(KT):
                    nc.tensor.matmul(vps[:], x_sb(b, k, i)[:, lsl],
                                     wv_sb[k][:],
                                     start=(k == 0), stop=(k == KT - 1))
                if prev is not None:
                    ctx_group(prev[0], lm, prev[1], prev[2])
                e = epool.tile([P, CHUNK], F16, tag="E", name="E")
                nc.scalar.activation(e[:], kps[:],
                                     mybir.ActivationFunctionType.Exp)
                E_t.append(e)
                v = epool.tile([P, HEADS * VW], F16, tag="vT", name="vT")
                v_view = v[:].rearrange("p (h e) -> p h e", e=VW)
                nc.vector.tensor_copy(
                    v_view[:, :, 0:DH],
                    vps[:].rearrange("p (h e) -> p h e", e=DH))
                nc.vector.tensor_scalar(
                    v_view[:, :, DH:DH + 2],
                    vps[:].rearrange("p (h e) -> p h e", e=DH)[:, :, 0:2],
                    0.0, 1.0, mybir.AluOpType.mult, mybir.AluOpType.add)
                vT_t.append(v)
            if y_batch is not None:
                y_chunk(y_batch, i)
            prev = (i, E_t, vT_t)
        for p in range(NPAIR):
            ctx_group(prev[0], p, prev[1], prev[2])
            if trailing_hook is not None:
                trailing_hook(p)
        return ctx_acc

    tmp_sb = {}

    def fold_pair(b, ctx_acc, p):
        """ctx~ = ctx/rowsum; tmp_h = ctx~_h^T @ wq_h (pair p)."""
        acc = ctx_acc[p]
        nc.vector.reciprocal(acc[0:DH, DH:DH + 1], acc[0:DH, DH:DH + 1])
        nc.vector.reciprocal(acc[DH:P, 2 * VW - 2:2 * VW - 1],
                             acc[DH:P, 2 * VW - 2:2 * VW - 1])
        ctxn = fpool.tile([P, DH], F16, tag=f"ctxn{p}", name=f"ctxn{p}")
        nc.vector.tensor_scalar_mul(ctxn[0:DH, :], acc[0:DH, 0:DH],
                                    acc[0:DH, DH:DH + 1])
        nc.vector.tensor_scalar_mul(ctxn[DH:P, :], acc[DH:P, VW:VW + DH],
                                    acc[DH:P, 2 * VW - 2:2 * VW - 1])
        tps = psy.tile([P, CHUNK], F32, tag="yps", name="yps")
        nc.tensor.matmul(tps[0:DH, :], ctxn[0:DH, :], wq_sb[p][0:DH, :],
                         start=True, stop=True, tile_position=(0, 0))
        nc.tensor.matmul(tps[DH:P, :], ctxn[DH:P, :], wq_sb[p][DH:P, :],
                         start=True, stop=True, tile_position=(DH, DH))
        t = fpool.tile([P, CHUNK], F16, tag=f"tmp{p}", name=f"tmp{p}")
        nc.scalar.copy(t[:], tps[:])
        tmp_sb[p] = t

    def fold_wyT(b):
        """WyT[m] = sum_p tmp-pair contraction with w_out^T."""
        for m in range(MT):
            wps = psy.tile([P, CHUNK], F32, tag="yps", name="yps")
            for p in range(NPAIR):
                nc.tensor.matmul(wps[:], tmp_sb[p][:, m * P:(m + 1) * P],
                                 woT_sb[p][:],
                                 start=(p == 0), stop=(p == NPAIR - 1))
            t = wypool.tile([P, DIM], F16, tag=f"wyt{b}_{m}",
                            name=f"wyt{b}_{m}")
            nc.vector.tensor_copy(t[:], wps[:])
            wyT_sb[(b, m)] = t

    yrow_sb = {}  # u -> [128, 4096] fp16 (reused across batches)

    def y_chunk(b, i):
        ls = slice(i * CHUNK, (i + 1) * CHUNK)
        for u in range(MT):
            yps = psy.tile([P, CHUNK], F32, tag="yps", name="yps")
            for k in range(KT):
                nc.tensor.matmul(yps[:],
                                 wyT_sb[(b, k)][:, u * P:(u + 1) * P],
                                 x_sb(b, k, i),
                                 start=(k == 0), stop=(k == KT - 1))
            if i == 0:
                yrow_sb[u] = ypool.tile([P, L], F16, tag=f"yrow{u}",
                                        name=f"yrow{u}")
            nc.vector.tensor_scalar_add(yrow_sb[u][:, ls], yps[:],
                                        bias_sb[:, u:u + 1])
            if i == NCHUNK // 2 - 1:
                nc.gpsimd.dma_start(
                    y_out[b, u * P:(u + 1) * P, 0:L // 2],
                    yrow_sb[u][:, 0:L // 2])
            elif i == NCHUNK - 1:
                nc.gpsimd.dma_start(
                    y_out[b, u * P:(u + 1) * P, L // 2:L],
                    yrow_sb[u][:, L // 2:L])

    def pass_Y(b):
        """y = WyT^T @ x + bias for batch b; DMA out as fp16."""
        for i in range(NCHUNK):
            y_chunk(b, i)

    acc_box = {}

    def hook0(p):
        fold_pair(0, acc_box[0], p)

    def hook1(p):
        fold_pair(1, acc_box[1], p)

    acc_box[0] = [cpool.tile([P, 2 * VW], F32, tag=f"ctxacc{p}",
                             name=f"ctxacc{p}") for p in range(NPAIR)]
    pass_A(0, y_batch=None, trailing_hook=hook0, ctx_acc=acc_box[0])
    fold_wyT(0)
    acc_box[1] = [cpool.tile([P, 2 * VW], F32, tag=f"ctxacc{p}",
                             name=f"ctxacc{p}") for p in range(NPAIR)]
    pass_A(1, y_batch=0, trailing_hook=hook1, ctx_acc=acc_box[1])
    fold_wyT(1)
    pass_Y(1)


def build_module():
    nc = bacc.Bacc("TRN2", target_bir_lowering=False, debug=False,
                   num_devices=NCORES)
    x_in = nc.dram_tensor("x", [BPC, DIM, L], F16, kind="ExternalInput")
    wkvT_in = nc.dram_tensor("w_kvT", [DIM, 2 * HIDDEN], F16,
                             kind="ExternalInput")
    wq_in = nc.dram_tensor("w_q", [HIDDEN, DIM], F16, kind="ExternalInput")
    woT_in = nc.dram_tensor("w_oT", [HIDDEN, DIM], F16, kind="ExternalInput")
    bias_in = nc.dram_tensor("bias", [P, MT], F32, kind="ExternalInput")
    y_out = nc.dram_tensor("y", [BPC, DIM, L], F16, kind="ExternalOutput")
    with tile.TileContext(nc) as tc:
        with ExitStack() as ctx:
            build_kernel(ctx, tc, x_in, wkvT_in, wq_in, woT_in, bias_in,
                         y_out)
    nc.compile()
    return nc


def make_in_maps(x, w_qkv, w_out, b_out):
    x = np.ascontiguousarray(x, dtype=np.float32).reshape(B, DIM, L)
    x16 = x.astype(np.float16)
    w_qkv = np.asarray(w_qkv, dtype=np.float32)
    wq = np.ascontiguousarray(w_qkv[0:HIDDEN]).astype(np.float16)
    wkvT = np.ascontiguousarray(
        np.concatenate([w_qkv[HIDDEN:2 * HIDDEN].T,
                        w_qkv[2 * HIDDEN:3 * HIDDEN].T], axis=1)
    ).astype(np.float16)
    woT = np.ascontiguousarray(
        np.asarray(w_out, dtype=np.float32).T).astype(np.float16)
    bias = np.ascontiguousarray(
        np.asarray(b_out, dtype=np.float32).reshape(MT, P).T)
    in_maps = []
    for c in range(NCORES):
        in_maps.append({
            "x": x16[c * BPC:(c + 1) * BPC],
            "w_kvT": wkvT,
            "w_q": wq,
            "w_oT": woT,
            "bias": bias,
        })
    return in_maps


_NC_CACHE = None


def kernel(x, w_qkv, w_out, b_out, *, trace=False, trace_kwargs=None):
    """Full inputs in, full output out. Shards batch across 8 NeuronCores."""
    global _NC_CACHE
    from concourse.bass_utils import run_bass_kernel_spmd

    if _NC_CACHE is None:
        _NC_CACHE = build_module()
    nc = _NC_CACHE

    in_maps = make_in_maps(x, w_qkv, w_out, b_out)
    kw = dict(trace_kwargs or {})
    res = run_bass_kernel_spmd(nc, in_maps, list(range(NCORES)),
                               trace=trace, **kw)
    y = np.empty((B, DIM, HGT, WID), dtype=np.float32)
    for c in range(NCORES):
        y[c * BPC:(c + 1) * BPC] = res.results[c]["y"].astype(
            np.float32).reshape(BPC, DIM, HGT, WID)
    kernel.last_results = res
    return y


# revision 10
# speedup vs baseline: 1.0006x; 1.0006x over previous
# Trainium2 Bass kernel for LinearAttention — v2 (q-path folded).
#
# Reference computation (per batch element b of 16):
#   qkv = w_qkv @ x[b]; q,k,v split into 8 heads x 64 dims
#   E = exp(k); ctx_h = (E_h/rowsum) @ v_h^T        # [64, 64]
#   y = w_out @ concat(ctx_h^T @ q_h) + b_out
#
# Key algebra: y = Wy @ x + b where
#   Wy = w_out @ blockdiag(ctx~_h^T) @ w_q   (per batch, [512, 512])
# so q never needs to be computed over l. Per batch:
#   Pass A: kT/vT projections (l on partitions), E = exp(kT), ctx
#           accumulation via head-pair matmuls with ones columns in vT
#           producing rowsums.
#   Fold:   ctx~ = ctx/rowsum; tmp_h = ctx~_h^T @ w_q_h;
#           WyT[c, y] = sum_h tmp_h^T-contract-w_outT  ([512, 512])
#   Pass Y: y = WyT^T-contract-x + bias; DMA out (fp16).
#
# All matmuls in fp16 (1 cycle/row at any N; fp32 PSUM accumulate).
# Data-parallel over batch: 16 batches / 8 cores = 2 per core.

import numpy as np
from contextlib import ExitStack

import concourse.bass as bass
import concourse.bacc as bacc
import concourse.mybir as mybir
import concourse.tile as tile

B, DIM, HGT, WID = 16, 512, 64, 64
L = HGT * WID            # 4096
HEADS, DH = 8, 64
HIDDEN = HEADS * DH      # 512
NCORES = 8
BPC = B // NCORES        # 2 batches per core
P = 128
CHUNK = 512
NCHUNK = L // CHUNK      # 8
KT = DIM // P            # 4 contraction tiles over channels
MT = DIM // P            # 4 output row tiles
LM = CHUNK // P          # 4 l-subtiles per chunk
NPAIR = HEADS // 2       # 4 head pairs
VW = DH + 2              # per-head vT width: 64 v cols + 2 ones cols

F32 = mybir.dt.float32
F16 = mybir.dt.float16


def build_kernel(ctx: ExitStack, tc: "tile.TileContext", x_in, wkvT_in, wq_in,
                 woT_in, bias_in, y_out):
    nc = tc.nc

    wpool = ctx.enter_context(tc.tile_pool(name="weights", bufs=1))
    xpool = ctx.enter_context(tc.tile_pool(name="xres", bufs=1))
    epool = ctx.enter_context(tc.tile_pool(name="ev", bufs=6))
    ypool = ctx.enter_context(tc.tile_pool(name="ysb", bufs=1))
    cpool = ctx.enter_context(tc.tile_pool(name="ctxacc", bufs=1))
    fpool = ctx.enter_context(tc.tile_pool(name="fold", bufs=1))
    wypool = ctx.enter_context(tc.tile_pool(name="wyt", bufs=1))
    psk = ctx.enter_context(tc.tile_pool(name="psk", bufs=2, space="PSUM"))
    psv = ctx.enter_context(tc.tile_pool(name="psv", bufs=2, space="PSUM"))
    psy = ctx.enter_context(tc.tile_pool(name="psy", bufs=2, space="PSUM"))
    psc = ctx.enter_context(tc.tile_pool(name="psc", bufs=2, space="PSUM"))

    # ---- k/v weights first (needed by the first matmuls), split into
    # k-half and v-half tiles so the first k matmuls wait on only 512KB ----
    dma_eng0 = [nc.sync, nc.scalar, nc.gpsimd]
    wk_sb, wv_sb = [], []
    for k in range(KT):
        t = wpool.tile([P, HIDDEN], F16, tag=f"wk{k}", name=f"wk{k}")
        dma_eng0[k % 3].dma_start(t[:], wkvT_in[k * P:(k + 1) * P, 0:HIDDEN])
        wk_sb.append(t)

    # ---- resident x: chunk-pair tiles [128, 1024] (2KB rows for DMA
    # efficiency, 256KB dependency granularity for fast start), issued
    # pair-major round-robin over the three DMA-capable engine queues ----
    dma_engines = [nc.sync, nc.scalar, nc.gpsimd]
    PAIRW = 2 * CHUNK
    xj_sb = {}  # (b, kt, j) -> [128, 1024] fp16, j = chunk pair

    def load_x(b, j):
        for k in range(KT):
            t = xpool.tile([P, PAIRW], F16, tag=f"x{b}_{k}_{j}",
                           name=f"x{b}_{k}_{j}")
            dma_engines[(k + j) % 3].dma_start(
                t[:], x_in[b, k * P:(k + 1) * P,
                           j * PAIRW:(j + 1) * PAIRW])
            xj_sb[(b, k, j)] = t

    load_x(0, 0)

    for k in range(KT):
        t = wpool.tile([P, HIDDEN], F16, tag=f"wv{k}", name=f"wv{k}")
        dma_eng0[(k + 1) % 3].dma_start(
            t[:], wkvT_in[k * P:(k + 1) * P, HIDDEN:2 * HIDDEN])
        wv_sb.append(t)
    for j in range(1, NCHUNK // 2):
        load_x(0, j)
    for j in range(NCHUNK // 2):
        load_x(1, j)

    def x_sb(b, k, i):
        return xj_sb[(b, k, i // 2)][:, (i % 2) * CHUNK:(i % 2 + 1) * CHUNK]

    yrow_sb = {}  # (b, u) -> [128, 4096] fp16
    for b in range(BPC):
        for u in range(MT):
            yrow_sb[(b, u)] = ypool.tile([P, L], F16, tag=f"yrow{b}_{u}",
                                         name=f"yrow{b}_{u}")

    # fold-time weights (not needed until after pass A)
    wq_sb = []    # [128 hid-d, 512 c] per pair tile (wq rows as-is)
    for p in range(NPAIR):
        t = wpool.tile([P, DIM], F16, tag=f"wq{p}", name=f"wq{p}")
        nc.sync.dma_start(t[:], wq_in[p * P:(p + 1) * P, :])
        wq_sb.append(t)
    woT_sb = []   # [128 hid-e, 512 y] per pair tile (w_out^T rows)
    for p in range(NPAIR):
        t = wpool.tile([P, DIM], F16, tag=f"wo{p}", name=f"wo{p}")
        nc.scalar.dma_start(t[:], woT_in[p * P:(p + 1) * P, :])
        woT_sb.append(t)
    bias_sb = wpool.tile([P, MT], F32, tag="bias", name="bias")
    nc.gpsimd.dma_start(bias_sb[:], bias_in[:])

    wyT_sb = {}  # (b, m) -> [128 c-sub, 512 y] fp16

    def pass_A(b, y_batch=None, trailing_hook=None, ctx_acc=None):
        """kT/vT projections, exp, ctx accumulation for batch b; optionally
        interleaves the y pass of a previous batch. ctx groups of chunk i-1
        are emitted between the kv/y groups of chunk i so their small-N
        matmuls' LDWEIGHTS hide under big matmuls (queue reorder window).
        trailing_hook(p) is called right after pair p's last ctx group so
        per-pair finalize work overlaps the remaining ctx groups."""
        def ctx_group(i, p, E_t, vT_t):
            pc = psc.tile([P, 2 * VW], F32, tag="ctx", name="ctx")
            for lm in range(LM):
                nc.tensor.matmul(
                    pc[:], E_t[lm][:, p * P:(p + 1) * P],
                    vT_t[lm][:, p * 2 * VW:(p + 1) * 2 * VW],
                    start=(lm == 0), stop=(lm == LM - 1))
            if i == 0:
                nc.vector.tensor_copy(ctx_acc[p][:], pc[:])
            else:
                nc.vector.tensor_add(ctx_acc[p][:], ctx_acc[p][:], pc[:])

        prev = None  # (i-1, E_t, vT_t)
        for i in range(NCHUNK):
            E_t, vT_t = [], []
            for lm in range(LM):
                lsl = slice(lm * P, (lm + 1) * P)
                kps = psk.tile([P, CHUNK], F32, tag="kps", name="kps")
                for k in range(KT):
                    nc.tensor.matmul(kps[:], x_sb(b, k, i)[:, lsl],
                                     wk_sb[k][:],
                                     start=(k == 0), stop=(k == KT - 1))
                vps = psv.tile([P, CHUNK], F32, tag="vps", name="vps")
                for k in range(KT):
                    nc.tensor.matmul(vps[:], x_sb(b, k, i)[:, lsl],
                                     wv_sb[k][:],
                                     start=(k == 0), stop=(k == KT - 1))
                if prev is not None:
                    ctx_group(prev[0], lm, prev[1], prev[2])
                e = epool.tile([P, CHUNK], F16, tag="E", name="E")
                nc.scalar.activation(e[:], kps[:],
                                     mybir.ActivationFunctionType.Exp)
                E_t.append(e)
                v = epool.tile([P, HEADS * VW], F16, tag="vT", name="vT")
                v_view = v[:].rearrange("p (h e) -> p h e", e=VW)
                nc.vector.tensor_copy(
                    v_view[:, :, 0:DH],
                    vps[:].rearrange("p (h e) -> p h e", e=DH))
                nc.vector.tensor_scalar(
                    v_view[:, :, DH:DH + 2],
                    vps[:].rearrange("p (h e) -> p h e", e=DH)[:, :, 0:2],
                    0.0, 1.0, mybir.AluOpType.mult, mybir.AluOpType.add)
                vT_t.append(v)
            if y_batch is not None:
                # u-strip halves: (u, chunks 0-3) on even i, (u, 4-8) on odd
                u, half = i // 2, i % 2
                y_ublock(y_batch, u, half * (NCHUNK // 2),
                         (half + 1) * (NCHUNK // 2))
            prev = (i, E_t, vT_t)
        for p in range(NPAIR):
            ctx_group(prev[0], p, prev[1], prev[2])
            if trailing_hook is not None:
                trailing_hook(p)
        return ctx_acc

    tmp_sb = {}

    def fold_pair(b, ctx_acc, p):
        """ctx~ = ctx/rowsum; tmp_h = ctx~_h^T @ wq_h (pair p)."""
        acc = ctx_acc[p]
        nc.vector.reciprocal(acc[0:DH, DH:DH + 1], acc[0:DH, DH:DH + 1])
        nc.vector.reciprocal(acc[DH:P, 2 * VW - 2:2 * VW - 1],
                             acc[DH:P, 2 * VW - 2:2 * VW - 1])
        ctxn = fpool.tile([P, DH], F16, tag=f"ctxn{p}", name=f"ctxn{p}")
        nc.vector.tensor_scalar_mul(ctxn[0:DH, :], acc[0:DH, 0:DH],
                                    acc[0:DH, DH:DH + 1])
        nc.vector.tensor_scalar_mul(ctxn[DH:P, :], acc[DH:P, VW:VW + DH],
                                    acc[DH:P, 2 * VW - 2:2 * VW - 1])
        tps = psy.tile([P, CHUNK], F32, tag="yps", name="yps")
        nc.tensor.matmul(tps[0:DH, :], ctxn[0:DH, :], wq_sb[p][0:DH, :],
                         start=True, stop=True, tile_position=(0, 0))
        nc.tensor.matmul(tps[DH:P, :], ctxn[DH:P, :], wq_sb[p][DH:P, :],
                         start=True, stop=True, tile_position=(DH, DH))
        t = fpool.tile([P, CHUNK], F16, tag=f"tmp{p}", name=f"tmp{p}")
        nc.scalar.copy(t[:], tps[:])
        tmp_sb[p] = t

    def fold_wyT(b):
        """WyT[m] = sum_p tmp-pair contraction with w_out^T."""
        for m in range(MT):
            wps = psy.tile([P, CHUNK], F32, tag="yps", name="yps")
            for p in range(NPAIR):
                nc.tensor.matmul(wps[:], tmp_sb[p][:, m * P:(m + 1) * P],
                                 woT_sb[p][:],
                                 start=(p == 0), stop=(p == NPAIR - 1))
            t = wypool.tile([P, DIM], F16, tag=f"wyt{b}_{m}",
                            name=f"wyt{b}_{m}")
            nc.vector.tensor_copy(t[:], wps[:])
            wyT_sb[(b, m)] = t

    def y_ublock(b, u, i0, i1):
        """y chunks [i0, i1) for output row strip u; DMA when the strip is
        complete so the store overlaps the remaining strips' compute."""
        yrow = yrow_sb[(b, u)]
        for i in range(i0, i1):
            ls = slice(i * CHUNK, (i + 1) * CHUNK)
            yps = psy.tile([P, CHUNK], F32, tag="yps", name="yps")
            for k in range(KT):
                nc.tensor.matmul(yps[:],
                                 wyT_sb[(b, k)][:, u * P:(u + 1) * P],
                                 x_sb(b, k, i),
                                 start=(k == 0), stop=(k == KT - 1))
            nc.vector.tensor_scalar_add(yrow[:, ls], yps[:],
                                        bias_sb[:, u:u + 1])
        if i1 == NCHUNK:
            nc.gpsimd.dma_start(y_out[b, u * P:(u + 1) * P, :], yrow[:])

    def pass_Y(b):
        """y = WyT^T @ x + bias for batch b; DMA out as fp16."""
        for u in range(MT):
            y_ublock(b, u, 0, NCHUNK)

    acc_box = {}

    def hook0(p):
        fold_pair(0, acc_box[0], p)

    def hook1(p):
        fold_pair(1, acc_box[1], p)

    acc_box[0] = [cpool.tile([P, 2 * VW], F32, tag=f"ctxacc{p}",
                             name=f"ctxacc{p}") for p in range(NPAIR)]
    pass_A(0, y_batch=None, trailing_hook=hook0, ctx_acc=acc_box[0])
    fold_wyT(0)
    acc_box[1] = [cpool.tile([P, 2 * VW], F32, tag=f"ctxacc{p}",
                             name=f"ctxacc{p}") for p in range(NPAIR)]
    pass_A(1, y_batch=0, trailing_hook=hook1, ctx_acc=acc_box[1])
    fold_wyT(1)
    pass_Y(1)


def build_module():
    nc = bacc.Bacc("TRN2", target_bir_lowering=False, debug=False,
                   num_devices=NCORES)
    x_in = nc.dram_tensor("x", [BPC, DIM, L], F16, kind="ExternalInput")
    wkvT_in = nc.dram_tensor("w_kvT", [DIM, 2 * HIDDEN], F16,
                             kind="ExternalInput")
    wq_in = nc.dram_tensor("w_q", [HIDDEN, DIM], F16, kind="ExternalInput")
    woT_in = nc.dram_tensor("w_oT", [HIDDEN, DIM], F16, kind="ExternalInput")
    bias_in = nc.dram_tensor("bias", [P, MT], F32, kind="ExternalInput")
    y_out = nc.dram_tensor("y", [BPC, DIM, L], F16, kind="ExternalOutput")
    with tile.TileContext(nc) as tc:
        with ExitStack() as ctx:
            build_kernel(ctx, tc, x_in, wkvT_in, wq_in, woT_in, bias_in,
                         y_out)
    nc.compile()
    return nc


def make_in_maps(x, w_qkv, w_out, b_out):
    x = np.ascontiguousarray(x, dtype=np.float32).reshape(B, DIM, L)
    x16 = x.astype(np.float16)
    w_qkv = np.asarray(w_qkv, dtype=np.float32)
    wq = np.ascontiguousarray(w_qkv[0:HIDDEN]).astype(np.float16)
    wkvT = np.ascontiguousarray(
        np.concatenate([w_qkv[HIDDEN:2 * HIDDEN].T,
                        w_qkv[2 * HIDDEN:3 * HIDDEN].T], axis=1)
    ).astype(np.float16)
    woT = np.ascontiguousarray(
        np.asarray(w_out, dtype=np.float32).T).astype(np.float16)
    bias = np.ascontiguousarray(
        np.asarray(b_out, dtype=np.float32).reshape(MT, P).T)
    in_maps = []
    for c in range(NCORES):
        in_maps.append({
            "x": x16[c * BPC:(c + 1) * BPC],
            "w_kvT": wkvT,
            "w_q": wq,
            "w_oT": woT,
            "bias": bias,
        })
    return in_maps


_NC_CACHE = None


def kernel(x, w_qkv, w_out, b_out, *, trace=False, trace_kwargs=None):
    """Full inputs in, full output out. Shards batch across 8 NeuronCores."""
    global _NC_CACHE
    from concourse.bass_utils import run_bass_kernel_spmd

    if _NC_CACHE is None:
        _NC_CACHE = build_module()
    nc = _NC_CACHE

    in_maps = make_in_maps(x, w_qkv, w_out, b_out)
    kw = dict(trace_kwargs or {})
    res = run_bass_kernel_spmd(nc, in_maps, list(range(NCORES)),
                               trace=trace, **kw)
    y = np.empty((B, DIM, HGT, WID), dtype=np.float32)
    for c in range(NCORES):
        y[c * BPC:(c + 1) * BPC] = res.results[c]["y"].astype(
            np.float32).reshape(BPC, DIM, HGT, WID)
    kernel.last_results = res
    return y


# revision 11
# speedup vs baseline: 1.0619x; 1.0612x over previous
# Trainium2 Bass kernel for LinearAttention — v2 (q-path folded).
#
# Reference computation (per batch element b of 16):
#   qkv = w_qkv @ x[b]; q,k,v split into 8 heads x 64 dims
#   E = exp(k); ctx_h = (E_h/rowsum) @ v_h^T        # [64, 64]
#   y = w_out @ concat(ctx_h^T @ q_h) + b_out
#
# Key algebra: y = Wy @ x + b where
#   Wy = w_out @ blockdiag(ctx~_h^T) @ w_q   (per batch, [512, 512])
# so q never needs to be computed over l. Per batch:
#   Pass A: kT/vT projections (l on partitions), E = exp(kT), ctx
#           accumulation via head-pair matmuls with ones columns in vT
#           producing rowsums.
#   Fold:   ctx~ = ctx/rowsum; tmp_h = ctx~_h^T @ w_q_h;
#           WyT[c, y] = sum_h tmp_h^T-contract-w_outT  ([512, 512])
#   Pass Y: y = WyT^T-contract-x + bias; DMA out (fp16).
#
# All matmuls in fp16 (1 cycle/row at any N; fp32 PSUM accumulate).
# Data-parallel over batch: 16 batches / 8 cores = 2 per core.

import numpy as np
from contextlib import ExitStack

import concourse.bass as bass
import concourse.bacc as bacc
import concourse.mybir as mybir
import concourse.tile as tile

B, DIM, HGT, WID = 16, 512, 64, 64
L = HGT * WID            # 4096
HEADS, DH = 8, 64
HIDDEN = HEADS * DH      # 512
NCORES = 8
BPC = B // NCORES        # 2 batches per core
P = 128
CHUNK = 512
NCHUNK = L // CHUNK      # 8
KT = DIM // P            # 4 contraction tiles over channels
MT = DIM // P            # 4 output row tiles
LM = CHUNK // P          # 4 l-subtiles per chunk
NPAIR = HEADS // 2       # 4 head pairs
VW = DH + 2              # per-head vT width: 64 v cols + 2 ones cols

F32 = mybir.dt.float32
F16 = mybir.dt.float16


def build_kernel(ctx: ExitStack, tc: "tile.TileContext", x_in, wkvT_in, wq_in,
                 woT_in, bias_in, y_out):
    nc = tc.nc

    wpool = ctx.enter_context(tc.tile_pool(name="weights", bufs=1))
    xpool = ctx.enter_context(tc.tile_pool(name="xres", bufs=1))
    epool = ctx.enter_context(tc.tile_pool(name="ev", bufs=6))
    ypool = ctx.enter_context(tc.tile_pool(name="ysb", bufs=1))
    cpool = ctx.enter_context(tc.tile_pool(name="ctxacc", bufs=1))
    fpool = ctx.enter_context(tc.tile_pool(name="fold", bufs=1))
    wypool = ctx.enter_context(tc.tile_pool(name="wyt", bufs=1))
    psk = ctx.enter_context(tc.tile_pool(name="psk", bufs=2, space="PSUM"))
    psv = ctx.enter_context(tc.tile_pool(name="psv", bufs=2, space="PSUM"))
    psy = ctx.enter_context(tc.tile_pool(name="psy", bufs=2, space="PSUM"))
    psc = ctx.enter_context(tc.tile_pool(name="psc", bufs=2, space="PSUM"))

    # ---- input loads. Queue = issuing engine; the Scalar queue also runs
    # the exp activations, so it gets only a bounded number of early
    # dispatches (DMA dispatch instructions block the engine queue on ring
    # flow control). x(b1) is emitted mid-pass-A so its dispatches queue
    # behind batch 0's critical work, not ahead of it. ----
    wk_sb, wv_sb = [], []
    for k in range(KT):
        wk_sb.append(wpool.tile([P, HIDDEN], F16, tag=f"wk{k}",
                                name=f"wk{k}"))
        wv_sb.append(wpool.tile([P, HIDDEN], F16, tag=f"wv{k}",
                                name=f"wv{k}"))
    PAIRW = 2 * CHUNK
    xj_sb = {}  # (b, kt, j) -> [128, 1024] fp16, j = chunk pair
    for b in range(BPC):
        for j in range(NCHUNK // 2):
            for k in range(KT):
                xj_sb[(b, k, j)] = xpool.tile(
                    [P, PAIRW], F16, tag=f"x{b}_{k}_{j}", name=f"x{b}_{k}_{j}")

    def dma_w(eng, t, src_ap):
        eng.dma_start(t[:], src_ap)

    def dma_x(eng, b, k, j):
        eng.dma_start(xj_sb[(b, k, j)][:],
                      x_in[b, k * P:(k + 1) * P,
                           j * PAIRW:(j + 1) * PAIRW])

    # startup-critical loads, explicitly laid out per queue
    dma_w(nc.scalar, wk_sb[0], wkvT_in[0:P, 0:HIDDEN])
    dma_w(nc.scalar, wk_sb[3], wkvT_in[3 * P:4 * P, 0:HIDDEN])
    dma_x(nc.scalar, 0, 1, 0)          # scalar stops here: exp comes next
    dma_x(nc.sync, 0, 0, 0)
    dma_x(nc.sync, 0, 3, 0)
    dma_w(nc.gpsimd, wk_sb[1], wkvT_in[P:2 * P, 0:HIDDEN])
    dma_w(nc.gpsimd, wk_sb[2], wkvT_in[2 * P:3 * P, 0:HIDDEN])
    dma_x(nc.gpsimd, 0, 2, 0)
    dma_w(nc.sync, wv_sb[0], wkvT_in[0:P, HIDDEN:2 * HIDDEN])
    dma_w(nc.gpsimd, wv_sb[1], wkvT_in[P:2 * P, HIDDEN:2 * HIDDEN])
    dma_w(nc.sync, wv_sb[2], wkvT_in[2 * P:3 * P, HIDDEN:2 * HIDDEN])
    dma_w(nc.gpsimd, wv_sb[3], wkvT_in[3 * P:4 * P, HIDDEN:2 * HIDDEN])
    for j in range(1, NCHUNK // 2):
        for k in range(KT):
            dma_x(nc.sync if (k + j) % 2 == 0 else nc.gpsimd, 0, k, j)

    def load_x_batch1():
        for j in range(NCHUNK // 2):
            for k in range(KT):
                dma_x(nc.sync if (k + j) % 2 == 0 else nc.gpsimd, 1, k, j)

    def x_sb(b, k, i):
        return xj_sb[(b, k, i // 2)][:, (i % 2) * CHUNK:(i % 2 + 1) * CHUNK]

    yrow_sb = {}  # (b, u) -> [128, 4096] fp16
    for b in range(BPC):
        for u in range(MT):
            yrow_sb[(b, u)] = ypool.tile([P, L], F16, tag=f"yrow{b}_{u}",
                                         name=f"yrow{b}_{u}")

    # fold-time weights (not needed until after pass A)
    wq_sb = []    # [128 hid-d, 512 c] per pair tile (wq rows as-is)
    for p in range(NPAIR):
        t = wpool.tile([P, DIM], F16, tag=f"wq{p}", name=f"wq{p}")
        nc.sync.dma_start(t[:], wq_in[p * P:(p + 1) * P, :])
        wq_sb.append(t)
    woT_sb = []   # [128 hid-e, 512 y] per pair tile (w_out^T rows)
    for p in range(NPAIR):
        t = wpool.tile([P, DIM], F16, tag=f"wo{p}", name=f"wo{p}")
        nc.scalar.dma_start(t[:], woT_in[p * P:(p + 1) * P, :])
        woT_sb.append(t)
    bias_sb = wpool.tile([P, MT], F32, tag="bias", name="bias")
    nc.gpsimd.dma_start(bias_sb[:], bias_in[:])

    wyT_sb = {}  # (b, m) -> [128 c-sub, 512 y] fp16

    def pass_A(b, y_batch=None, trailing_hook=None, ctx_acc=None):
        """kT/vT projections, exp, ctx accumulation for batch b; optionally
        interleaves the y pass of a previous batch. ctx groups of chunk i-1
        are emitted between the kv/y groups of chunk i so their small-N
        matmuls' LDWEIGHTS hide under big matmuls (queue reorder window).
        trailing_hook(p) is called right after pair p's last ctx group so
        per-pair finalize work overlaps the remaining ctx groups."""
        def ctx_group(i, p, E_t, vT_t):
            pc = psc.tile([P, 2 * VW], F32, tag="ctx", name="ctx")
            for lm in range(LM):
                nc.tensor.matmul(
                    pc[:], E_t[lm][:, p * P:(p + 1) * P],
                    vT_t[lm][:, p * 2 * VW:(p + 1) * 2 * VW],
                    start=(lm == 0), stop=(lm == LM - 1))
            if i == 0:
                nc.vector.tensor_copy(ctx_acc[p][:], pc[:])
            else:
                nc.vector.tensor_add(ctx_acc[p][:], ctx_acc[p][:], pc[:])

        prev = None  # (i-1, E_t, vT_t)
        for i in range(NCHUNK):
            E_t, vT_t = [], []
            for lm in range(LM):
                lsl = slice(lm * P, (lm + 1) * P)
                kps = psk.tile([P, CHUNK], F32, tag="kps", name="kps")
                for k in range(KT):
                    nc.tensor.matmul(kps[:], x_sb(b, k, i)[:, lsl],
                                     wk_sb[k][:],
                                     start=(k == 0), stop=(k == KT - 1))
                vps = psv.tile([P, CHUNK], F32, tag="vps", name="vps")
                for k in range(KT):
                    nc.tensor.matmul(vps[:], x_sb(b, k, i)[:, lsl],
                                     wv_sb[k][:],
                                     start=(k == 0), stop=(k == KT - 1))
                if prev is not None:
                    ctx_group(prev[0], lm, prev[1], prev[2])
                e = epool.tile([P, CHUNK], F16, tag="E", name="E")
                nc.scalar.activation(e[:], kps[:],
                                     mybir.ActivationFunctionType.Exp)
                E_t.append(e)
                v = epool.tile([P, HEADS * VW], F16, tag="vT", name="vT")
                v_view = v[:].rearrange("p (h e) -> p h e", e=VW)
                nc.vector.tensor_copy(
                    v_view[:, :, 0:DH],
                    vps[:].rearrange("p (h e) -> p h e", e=DH))
                nc.vector.tensor_scalar(
                    v_view[:, :, DH:DH + 2],
                    vps[:].rearrange("p (h e) -> p h e", e=DH)[:, :, 0:2],
                    0.0, 1.0, mybir.AluOpType.mult, mybir.AluOpType.add)
                vT_t.append(v)
            if y_batch is not None:
                # u-strip halves: (u, chunks 0-3) on even i, (u, 4-8) on odd
                u, half = i // 2, i % 2
                y_ublock(y_batch, u, half * (NCHUNK // 2),
                         (half + 1) * (NCHUNK // 2))
            if b == 0 and i == 1:
                load_x_batch1()
            prev = (i, E_t, vT_t)
        for p in range(NPAIR):
            ctx_group(prev[0], p, prev[1], prev[2])
            if trailing_hook is not None:
                trailing_hook(p)
        return ctx_acc

    tmp_sb = {}

    def fold_pair(b, ctx_acc, p):
        """ctx~ = ctx/rowsum; tmp_h = ctx~_h^T @ wq_h (pair p)."""
        acc = ctx_acc[p]
        nc.vector.reciprocal(acc[0:DH, DH:DH + 1], acc[0:DH, DH:DH + 1])
        nc.vector.reciprocal(acc[DH:P, 2 * VW - 2:2 * VW - 1],
                             acc[DH:P, 2 * VW - 2:2 * VW - 1])
        ctxn = fpool.tile([P, DH], F16, tag=f"ctxn{p}", name=f"ctxn{p}")
        nc.vector.tensor_scalar_mul(ctxn[0:DH, :], acc[0:DH, 0:DH],
                                    acc[0:DH, DH:DH + 1])
        nc.vector.tensor_scalar_mul(ctxn[DH:P, :], acc[DH:P, VW:VW + DH],
                                    acc[DH:P, 2 * VW - 2:2 * VW - 1])
        tps = psy.tile([P, CHUNK], F32, tag="yps", name="yps")
        nc.tensor.matmul(tps[0:DH, :], ctxn[0:DH, :], wq_sb[p][0:DH, :],
                         start=True, stop=True, tile_position=(0, 0))
        nc.tensor.matmul(tps[DH:P, :], ctxn[DH:P, :], wq_sb[p][DH:P, :],
                         start=True, stop=True, tile_position=(DH, DH))
        t = fpool.tile([P, CHUNK], F16, tag=f"tmp{p}", name=f"tmp{p}")
        nc.scalar.copy(t[:], tps[:])
        tmp_sb[p] = t

    def fold_wyT(b):
        """WyT[m] = sum_p tmp-pair contraction with w_out^T."""
        for m in range(MT):
            wps = psy.tile([P, CHUNK], F32, tag="yps", name="yps")
            for p in range(NPAIR):
                nc.tensor.matmul(wps[:], tmp_sb[p][:, m * P:(m + 1) * P],
                                 woT_sb[p][:],
                                 start=(p == 0), stop=(p == NPAIR - 1))
            t = wypool.tile([P, DIM], F16, tag=f"wyt{b}_{m}",
                            name=f"wyt{b}_{m}")
            nc.vector.tensor_copy(t[:], wps[:])
            wyT_sb[(b, m)] = t

    def y_ublock(b, u, i0, i1):
        """y chunks [i0, i1) for output row strip u; DMA when the strip is
        complete so the store overlaps the remaining strips' compute."""
        yrow = yrow_sb[(b, u)]
        for i in range(i0, i1):
            ls = slice(i * CHUNK, (i + 1) * CHUNK)
            yps = psy.tile([P, CHUNK], F32, tag="yps", name="yps")
            for k in range(KT):
                nc.tensor.matmul(yps[:],
                                 wyT_sb[(b, k)][:, u * P:(u + 1) * P],
                                 x_sb(b, k, i),
                                 start=(k == 0), stop=(k == KT - 1))
            nc.vector.tensor_scalar_add(yrow[:, ls], yps[:],
                                        bias_sb[:, u:u + 1])
        if i1 == NCHUNK:
            if b == BPC - 1 and u == MT - 1:
                nc.gpsimd.dma_start(
                    y_out[b, u * P:(u + 1) * P, 0:7 * CHUNK],
                    yrow[:, 0:7 * CHUNK])
            else:
                nc.gpsimd.dma_start(y_out[b, u * P:(u + 1) * P, :], yrow[:])

    def pass_Y(b):
        """y = WyT^T @ x + bias for batch b; DMA out as fp16."""
        for u in range(MT):
            y_ublock(b, u, 0, NCHUNK)
        nc.gpsimd.dma_start(
            y_out[b, (MT - 1) * P:MT * P, 7 * CHUNK:L],
            yrow_sb[(b, MT - 1)][:, 7 * CHUNK:L])

    acc_box = {}

    def hook0(p):
        fold_pair(0, acc_box[0], p)

    def hook1(p):
        fold_pair(1, acc_box[1], p)

    acc_box[0] = [cpool.tile([P, 2 * VW], F32, tag=f"ctxacc{p}",
                             name=f"ctxacc{p}") for p in range(NPAIR)]
    pass_A(0, y_batch=None, trailing_hook=hook0, ctx_acc=acc_box[0])
    fold_wyT(0)
    acc_box[1] = [cpool.tile([P, 2 * VW], F32, tag=f"ctxacc{p}",
                             name=f"ctxacc{p}") for p in range(NPAIR)]
    pass_A(1, y_batch=0, trailing_hook=hook1, ctx_acc=acc_box[1])
    fold_wyT(1)
    pass_Y(1)


def build_module():
    nc = bacc.Bacc("TRN2", target_bir_lowering=False, debug=False,
                   num_devices=NCORES)
    x_in = nc.dram_tensor("x", [BPC, DIM, L], F16, kind="ExternalInput")
    wkvT_in = nc.dram_tensor("w_kvT", [DIM, 2 * HIDDEN], F16,
                             kind="ExternalInput")
    wq_in = nc.dram_tensor("w_q", [HIDDEN, DIM], F16, kind="ExternalInput")
    woT_in = nc.dram_tensor("w_oT", [HIDDEN, DIM], F16, kind="ExternalInput")
    bias_in = nc.dram_tensor("bias", [P, MT], F32, kind="ExternalInput")
    y_out = nc.dram_tensor("y", [BPC, DIM, L], F16, kind="ExternalOutput")
    with tile.TileContext(nc) as tc:
        with ExitStack() as ctx:
            build_kernel(ctx, tc, x_in, wkvT_in, wq_in, woT_in, bias_in,
                         y_out)
    nc.compile()
    return nc


def make_in_maps(x, w_qkv, w_out, b_out):
    x = np.ascontiguousarray(x, dtype=np.float32).reshape(B, DIM, L)
    x16 = x.astype(np.float16)
    w_qkv = np.asarray(w_qkv, dtype=np.float32)
    wq = np.ascontiguousarray(w_qkv[0:HIDDEN]).astype(np.float16)
    wkvT = np.ascontiguousarray(
        np.concatenate([w_qkv[HIDDEN:2 * HIDDEN].T,
                        w_qkv[2 * HIDDEN:3 * HIDDEN].T], axis=1)
    ).astype(np.float16)
    woT = np.ascontiguousarray(
        np.asarray(w_out, dtype=np.float32).T).astype(np.float16)
    bias = np.ascontiguousarray(
        np.asarray(b_out, dtype=np.float32).reshape(MT, P).T)
    in_maps = []
    for c in range(NCORES):
        in_maps.append({
            "x": x16[c * BPC:(c + 1) * BPC],
            "w_kvT": wkvT,
            "w_q": wq,
            "w_oT": woT,
            "bias": bias,
        })
    return in_maps


_NC_CACHE = None


def kernel(x, w_qkv, w_out, b_out, *, trace=False, trace_kwargs=None):
    """Full inputs in, full output out. Shards batch across 8 NeuronCores."""
    global _NC_CACHE
    from concourse.bass_utils import run_bass_kernel_spmd

    if _NC_CACHE is None:
        _NC_CACHE = build_module()
    nc = _NC_CACHE

    in_maps = make_in_maps(x, w_qkv, w_out, b_out)
    kw = dict(trace_kwargs or {})
    res = run_bass_kernel_spmd(nc, in_maps, list(range(NCORES)),
                               trace=trace, **kw)
    y = np.empty((B, DIM, HGT, WID), dtype=np.float32)
    for c in range(NCORES):
        y[c * BPC:(c + 1) * BPC] = res.results[c]["y"].astype(
            np.float32).reshape(BPC, DIM, HGT, WID)
    kernel.last_results = res
    return y


# revision 12
# speedup vs baseline: 1.0732x; 1.0107x over previous
# Trainium2 Bass kernel for LinearAttention — v2 (q-path folded).
#
# Reference computation (per batch element b of 16):
#   qkv = w_qkv @ x[b]; q,k,v split into 8 heads x 64 dims
#   E = exp(k); ctx_h = (E_h/rowsum) @ v_h^T        # [64, 64]
#   y = w_out @ concat(ctx_h^T @ q_h) + b_out
#
# Key algebra: y = Wy @ x + b where
#   Wy = w_out @ blockdiag(ctx~_h^T) @ w_q   (per batch, [512, 512])
# so q never needs to be computed over l. Per batch:
#   Pass A: kT/vT projections (l on partitions), E = exp(kT), ctx
#           accumulation via head-pair matmuls with ones columns in vT
#           producing rowsums.
#   Fold:   ctx~ = ctx/rowsum; tmp_h = ctx~_h^T @ w_q_h;
#           WyT[c, y] = sum_h tmp_h^T-contract-w_outT  ([512, 512])
#   Pass Y: y = WyT^T-contract-x + bias; DMA out (fp16).
#
# All matmuls in fp16 (1 cycle/row at any N; fp32 PSUM accumulate).
# Data-parallel over batch: 16 batches / 8 cores = 2 per core.

import numpy as np
from contextlib import ExitStack

import concourse.bass as bass
import concourse.bacc as bacc
import concourse.mybir as mybir
import concourse.tile as tile

B, DIM, HGT, WID = 16, 512, 64, 64
L = HGT * WID            # 4096
HEADS, DH = 8, 64
HIDDEN = HEADS * DH      # 512
NCORES = 8
BPC = B // NCORES        # 2 batches per core
P = 128
CHUNK = 512
NCHUNK = L // CHUNK      # 8
KT = DIM // P            # 4 contraction tiles over channels
MT = DIM // P            # 4 output row tiles
LM = CHUNK // P          # 4 l-subtiles per chunk
NPAIR = HEADS // 2       # 4 head pairs
VW = DH + 2              # per-head vT width: 64 v cols + 2 ones cols

F32 = mybir.dt.float32
F16 = mybir.dt.float16


def build_kernel(ctx: ExitStack, tc: "tile.TileContext", x_in, wkvT_in, wq_in,
                 woT_in, bias_in, y_out):
    nc = tc.nc

    wpool = ctx.enter_context(tc.tile_pool(name="weights", bufs=1))
    xpool = ctx.enter_context(tc.tile_pool(name="xres", bufs=1))
    epool = ctx.enter_context(tc.tile_pool(name="ev", bufs=6))
    ypool = ctx.enter_context(tc.tile_pool(name="ysb", bufs=1))
    cpool = ctx.enter_context(tc.tile_pool(name="ctxacc", bufs=1))
    fpool = ctx.enter_context(tc.tile_pool(name="fold", bufs=1))
    wypool = ctx.enter_context(tc.tile_pool(name="wyt", bufs=1))
    psk = ctx.enter_context(tc.tile_pool(name="psk", bufs=2, space="PSUM"))
    psv = ctx.enter_context(tc.tile_pool(name="psv", bufs=2, space="PSUM"))
    psy = ctx.enter_context(tc.tile_pool(name="psy", bufs=2, space="PSUM"))
    psc = ctx.enter_context(tc.tile_pool(name="psc", bufs=2, space="PSUM"))

    # ---- input loads. Queue = issuing engine; the Scalar queue also runs
    # the exp activations, so it gets only a bounded number of early
    # dispatches (DMA dispatch instructions block the engine queue on ring
    # flow control). x(b1) is emitted mid-pass-A so its dispatches queue
    # behind batch 0's critical work, not ahead of it. ----
    wk_sb, wv_sb = [], []
    for k in range(KT):
        wk_sb.append(wpool.tile([P, HIDDEN], F16, tag=f"wk{k}",
                                name=f"wk{k}"))
        wv_sb.append(wpool.tile([P, HIDDEN], F16, tag=f"wv{k}",
                                name=f"wv{k}"))
    PAIRW = 2 * CHUNK
    xj_sb = {}  # (b, kt, j) -> [128, 1024] fp16, j = chunk pair
    for b in range(BPC):
        for j in range(NCHUNK // 2):
            for k in range(KT):
                xj_sb[(b, k, j)] = xpool.tile(
                    [P, PAIRW], F16, tag=f"x{b}_{k}_{j}", name=f"x{b}_{k}_{j}")

    def dma_w(eng, t, src_ap):
        eng.dma_start(t[:], src_ap)

    def dma_x(eng, b, k, j):
        eng.dma_start(xj_sb[(b, k, j)][:],
                      x_in[b, k * P:(k + 1) * P,
                           j * PAIRW:(j + 1) * PAIRW])

    # startup-critical loads, explicitly laid out per queue
    dma_w(nc.scalar, wk_sb[0], wkvT_in[0:P, 0:HIDDEN])
    dma_w(nc.scalar, wk_sb[3], wkvT_in[3 * P:4 * P, 0:HIDDEN])
    dma_x(nc.scalar, 0, 1, 0)          # scalar stops here: exp comes next
    dma_x(nc.sync, 0, 0, 0)
    dma_x(nc.sync, 0, 3, 0)
    dma_w(nc.gpsimd, wk_sb[1], wkvT_in[P:2 * P, 0:HIDDEN])
    dma_w(nc.gpsimd, wk_sb[2], wkvT_in[2 * P:3 * P, 0:HIDDEN])
    dma_x(nc.gpsimd, 0, 2, 0)
    dma_w(nc.sync, wv_sb[0], wkvT_in[0:P, HIDDEN:2 * HIDDEN])
    dma_w(nc.gpsimd, wv_sb[1], wkvT_in[P:2 * P, HIDDEN:2 * HIDDEN])
    dma_w(nc.sync, wv_sb[2], wkvT_in[2 * P:3 * P, HIDDEN:2 * HIDDEN])
    dma_w(nc.gpsimd, wv_sb[3], wkvT_in[3 * P:4 * P, HIDDEN:2 * HIDDEN])
    for j in range(1, NCHUNK // 2):
        for k in range(KT):
            dma_x(nc.sync if (k + j) % 2 == 0 else nc.gpsimd, 0, k, j)

    def load_x_batch1():
        for j in range(NCHUNK // 2):
            for k in range(KT):
                dma_x(nc.sync if (k + j) % 2 == 0 else nc.gpsimd, 1, k, j)

    def x_sb(b, k, i):
        return xj_sb[(b, k, i // 2)][:, (i % 2) * CHUNK:(i % 2 + 1) * CHUNK]

    yrow_sb = {}  # (b, u) -> [128, 4096] fp16
    for b in range(BPC):
        for u in range(MT):
            yrow_sb[(b, u)] = ypool.tile([P, L], F16, tag=f"yrow{b}_{u}",
                                         name=f"yrow{b}_{u}")

    # fold-time weights: tiles declared here, DMAs emitted mid-pass-A (they
    # are not needed until fold, and must not clog any queue at startup)
    wq_sb = [wpool.tile([P, DIM], F16, tag=f"wq{p}", name=f"wq{p}")
             for p in range(NPAIR)]
    woT_sb = [wpool.tile([P, DIM], F16, tag=f"wo{p}", name=f"wo{p}")
              for p in range(NPAIR)]
    bias_sb = wpool.tile([P, MT], F32, tag="bias", name="bias")

    def load_fold_weights():
        for p in range(NPAIR):
            nc.sync.dma_start(wq_sb[p][:], wq_in[p * P:(p + 1) * P, :])
            nc.gpsimd.dma_start(woT_sb[p][:], woT_in[p * P:(p + 1) * P, :])
        nc.gpsimd.dma_start(bias_sb[:], bias_in[:])

    wyT_sb = {}  # (b, m) -> [128 c-sub, 512 y] fp16

    def pass_A(b, y_batch=None, trailing_hook=None, ctx_acc=None):
        """kT/vT projections, exp, ctx accumulation for batch b; optionally
        interleaves the y pass of a previous batch. ctx groups of chunk i-1
        are emitted between the kv/y groups of chunk i so their small-N
        matmuls' LDWEIGHTS hide under big matmuls (queue reorder window).
        trailing_hook(p) is called right after pair p's last ctx group so
        per-pair finalize work overlaps the remaining ctx groups."""
        def ctx_group(i, p, E_t, vT_t):
            pc = psc.tile([P, 2 * VW], F32, tag="ctx", name="ctx")
            for lm in range(LM):
                nc.tensor.matmul(
                    pc[:], E_t[lm][:, p * P:(p + 1) * P],
                    vT_t[lm][:, p * 2 * VW:(p + 1) * 2 * VW],
                    start=(lm == 0), stop=(lm == LM - 1))
            if i == 0:
                nc.vector.tensor_copy(ctx_acc[p][:], pc[:])
            else:
                nc.vector.tensor_add(ctx_acc[p][:], ctx_acc[p][:], pc[:])

        prev = None  # (i-1, E_t, vT_t)
        for i in range(NCHUNK):
            E_t, vT_t = [], []
            # all k-groups first: the exp drains chase them with slack, so
            # the next chunk's kps allocation never stalls on ACT
            for lm in range(LM):
                lsl = slice(lm * P, (lm + 1) * P)
                kps = psk.tile([P, CHUNK], F32, tag="kps", name="kps")
                for k in range(KT):
                    nc.tensor.matmul(kps[:], x_sb(b, k, i)[:, lsl],
                                     wk_sb[k][:],
                                     start=(k == 0), stop=(k == KT - 1))
                e = epool.tile([P, CHUNK], F16, tag="E", name="E")
                nc.scalar.activation(e[:], kps[:],
                                     mybir.ActivationFunctionType.Exp)
                E_t.append(e)
            for lm in range(LM):
                lsl = slice(lm * P, (lm + 1) * P)
                vps = psv.tile([P, CHUNK], F32, tag="vps", name="vps")
                for k in range(KT):
                    nc.tensor.matmul(vps[:], x_sb(b, k, i)[:, lsl],
                                     wv_sb[k][:],
                                     start=(k == 0), stop=(k == KT - 1))
                if prev is not None:
                    ctx_group(prev[0], lm, prev[1], prev[2])
                v = epool.tile([P, HEADS * VW], F16, tag="vT", name="vT")
                v_view = v[:].rearrange("p (h e) -> p h e", e=VW)
                nc.vector.tensor_copy(
                    v_view[:, :, 0:DH],
                    vps[:].rearrange("p (h e) -> p h e", e=DH))
                nc.vector.tensor_scalar(
                    v_view[:, :, DH:DH + 2],
                    vps[:].rearrange("p (h e) -> p h e", e=DH)[:, :, 0:2],
                    0.0, 1.0, mybir.AluOpType.mult, mybir.AluOpType.add)
                vT_t.append(v)
            if y_batch is not None:
                # u-strip halves: (u, chunks 0-3) on even i, (u, 4-8) on odd
                u, half = i // 2, i % 2
                y_ublock(y_batch, u, half * (NCHUNK // 2),
                         (half + 1) * (NCHUNK // 2))
            if b == 0 and i == 1:
                load_x_batch1()
            if b == 0 and i == 3:
                load_fold_weights()
            prev = (i, E_t, vT_t)
        for p in range(NPAIR):
            ctx_group(prev[0], p, prev[1], prev[2])
            if trailing_hook is not None:
                trailing_hook(p)
        return ctx_acc

    tmp_sb = {}

    def fold_pair(b, ctx_acc, p):
        """ctx~ = ctx/rowsum; tmp_h = ctx~_h^T @ wq_h (pair p)."""
        acc = ctx_acc[p]
        nc.vector.reciprocal(acc[0:DH, DH:DH + 1], acc[0:DH, DH:DH + 1])
        nc.vector.reciprocal(acc[DH:P, 2 * VW - 2:2 * VW - 1],
                             acc[DH:P, 2 * VW - 2:2 * VW - 1])
        ctxn = fpool.tile([P, DH], F16, tag=f"ctxn{p}", name=f"ctxn{p}")
        nc.vector.tensor_scalar_mul(ctxn[0:DH, :], acc[0:DH, 0:DH],
                                    acc[0:DH, DH:DH + 1])
        nc.vector.tensor_scalar_mul(ctxn[DH:P, :], acc[DH:P, VW:VW + DH],
                                    acc[DH:P, 2 * VW - 2:2 * VW - 1])
        tps = psy.tile([P, CHUNK], F32, tag="yps", name="yps")
        nc.tensor.matmul(tps[0:DH, :], ctxn[0:DH, :], wq_sb[p][0:DH, :],
                         start=True, stop=True, tile_position=(0, 0))
        nc.tensor.matmul(tps[DH:P, :], ctxn[DH:P, :], wq_sb[p][DH:P, :],
                         start=True, stop=True, tile_position=(DH, DH))
        t = fpool.tile([P, CHUNK], F16, tag=f"tmp{p}", name=f"tmp{p}")
        nc.scalar.copy(t[:], tps[:])
        tmp_sb[p] = t

    def fold_wyT(b):
        """WyT[m] = sum_p tmp-pair contraction with w_out^T."""
        for m in range(MT):
            wps = psy.tile([P, CHUNK], F32, tag="yps", name="yps")
            for p in range(NPAIR):
                nc.tensor.matmul(wps[:], tmp_sb[p][:, m * P:(m + 1) * P],
                                 woT_sb[p][:],
                                 start=(p == 0), stop=(p == NPAIR - 1))
            t = wypool.tile([P, DIM], F16, tag=f"wyt{b}_{m}",
                            name=f"wyt{b}_{m}")
            nc.vector.tensor_copy(t[:], wps[:])
            wyT_sb[(b, m)] = t

    def y_ublock(b, u, i0, i1):
        """y chunks [i0, i1) for output row strip u; DMA when the strip is
        complete so the store overlaps the remaining strips' compute."""
        yrow = yrow_sb[(b, u)]
        for i in range(i0, i1):
            ls = slice(i * CHUNK, (i + 1) * CHUNK)
            yps = psy.tile([P, CHUNK], F32, tag="yps", name="yps")
            for k in range(KT):
                nc.tensor.matmul(yps[:],
                                 wyT_sb[(b, k)][:, u * P:(u + 1) * P],
                                 x_sb(b, k, i),
                                 start=(k == 0), stop=(k == KT - 1))
            nc.vector.tensor_scalar_add(yrow[:, ls], yps[:],
                                        bias_sb[:, u:u + 1])
            # fire the store in pieces as chunks complete so only the final
            # 256KB piece is exposed at the end of the kernel
            if i == 3:
                nc.gpsimd.dma_start(
                    y_out[b, u * P:(u + 1) * P, 0:4 * CHUNK],
                    yrow[:, 0:4 * CHUNK])
            elif i == 6:
                nc.gpsimd.dma_start(
                    y_out[b, u * P:(u + 1) * P, 4 * CHUNK:7 * CHUNK],
                    yrow[:, 4 * CHUNK:7 * CHUNK])
            elif i == 7:
                nc.gpsimd.dma_start(
                    y_out[b, u * P:(u + 1) * P, 7 * CHUNK:L],
                    yrow[:, 7 * CHUNK:L])

    def pass_Y(b):
        """y = WyT^T @ x + bias for batch b; DMA out as fp16."""
        for u in range(MT):
            y_ublock(b, u, 0, NCHUNK)

    acc_box = {}

    def hook0(p):
        fold_pair(0, acc_box[0], p)

    def hook1(p):
        fold_pair(1, acc_box[1], p)

    acc_box[0] = [cpool.tile([P, 2 * VW], F32, tag=f"ctxacc{p}",
                             name=f"ctxacc{p}") for p in range(NPAIR)]
    pass_A(0, y_batch=None, trailing_hook=hook0, ctx_acc=acc_box[0])
    fold_wyT(0)
    acc_box[1] = [cpool.tile([P, 2 * VW], F32, tag=f"ctxacc{p}",
                             name=f"ctxacc{p}") for p in range(NPAIR)]
    pass_A(1, y_batch=0, trailing_hook=hook1, ctx_acc=acc_box[1])
    fold_wyT(1)
    pass_Y(1)


def build_module():
    nc = bacc.Bacc("TRN2", target_bir_lowering=False, debug=False,
                   num_devices=NCORES)
    x_in = nc.dram_tensor("x", [BPC, DIM, L], F16, kind="ExternalInput")
    wkvT_in = nc.dram_tensor("w_kvT", [DIM, 2 * HIDDEN], F16,
                             kind="ExternalInput")
    wq_in = nc.dram_tensor("w_q", [HIDDEN, DIM], F16, kind="ExternalInput")
    woT_in = nc.dram_tensor("w_oT", [HIDDEN, DIM], F16, kind="ExternalInput")
    bias_in = nc.dram_tensor("bias", [P, MT], F32, kind="ExternalInput")
    y_out = nc.dram_tensor("y", [BPC, DIM, L], F16, kind="ExternalOutput")
    with tile.TileContext(nc) as tc:
        with ExitStack() as ctx:
            build_kernel(ctx, tc, x_in, wkvT_in, wq_in, woT_in, bias_in,
                         y_out)
    nc.compile()
    return nc


def make_in_maps(x, w_qkv, w_out, b_out):
    x = np.ascontiguousarray(x, dtype=np.float32).reshape(B, DIM, L)
    x16 = x.astype(np.float16)
    w_qkv = np.asarray(w_qkv, dtype=np.float32)
    wq = np.ascontiguousarray(w_qkv[0:HIDDEN]).astype(np.float16)
    wkvT = np.ascontiguousarray(
        np.concatenate([w_qkv[HIDDEN:2 * HIDDEN].T,
                        w_qkv[2 * HIDDEN:3 * HIDDEN].T], axis=1)
    ).astype(np.float16)
    woT = np.ascontiguousarray(
        np.asarray(w_out, dtype=np.float32).T).astype(np.float16)
    bias = np.ascontiguousarray(
        np.asarray(b_out, dtype=np.float32).reshape(MT, P).T)
    in_maps = []
    for c in range(NCORES):
        in_maps.append({
            "x": x16[c * BPC:(c + 1) * BPC],
            "w_kvT": wkvT,
            "w_q": wq,
            "w_oT": woT,
            "bias": bias,
        })
    return in_maps


_NC_CACHE = None


def kernel(x, w_qkv, w_out, b_out, *, trace=False, trace_kwargs=None):
    """Full inputs in, full output out. Shards batch across 8 NeuronCores."""
    global _NC_CACHE
    from concourse.bass_utils import run_bass_kernel_spmd

    if _NC_CACHE is None:
        _NC_CACHE = build_module()
    nc = _NC_CACHE

    in_maps = make_in_maps(x, w_qkv, w_out, b_out)
    kw = dict(trace_kwargs or {})
    res = run_bass_kernel_spmd(nc, in_maps, list(range(NCORES)),
                               trace=trace, **kw)
    y = np.empty((B, DIM, HGT, WID), dtype=np.float32)
    for c in range(NCORES):
        y[c * BPC:(c + 1) * BPC] = res.results[c]["y"].astype(
            np.float32).reshape(BPC, DIM, HGT, WID)
    kernel.last_results = res
    return y


# revision 13
# speedup vs baseline: 1.0826x; 1.0087x over previous
# Trainium2 Bass kernel for LinearAttention — v2 (q-path folded).
#
# Reference computation (per batch element b of 16):
#   qkv = w_qkv @ x[b]; q,k,v split into 8 heads x 64 dims
#   E = exp(k); ctx_h = (E_h/rowsum) @ v_h^T        # [64, 64]
#   y = w_out @ concat(ctx_h^T @ q_h) + b_out
#
# Key algebra: y = Wy @ x + b where
#   Wy = w_out @ blockdiag(ctx~_h^T) @ w_q   (per batch, [512, 512])
# so q never needs to be computed over l. Per batch:
#   Pass A: kT/vT projections (l on partitions), E = exp(kT), ctx
#           accumulation via head-pair matmuls with ones columns in vT
#           producing rowsums.
#   Fold:   ctx~ = ctx/rowsum; tmp_h = ctx~_h^T @ w_q_h;
#           WyT[c, y] = sum_h tmp_h^T-contract-w_outT  ([512, 512])
#   Pass Y: y = WyT^T-contract-x + bias; DMA out (fp16).
#
# All matmuls in fp16 (1 cycle/row at any N; fp32 PSUM accumulate).
# Data-parallel over batch: 16 batches / 8 cores = 2 per core.

import numpy as np
from contextlib import ExitStack

import concourse.bass as bass
import concourse.bacc as bacc
import concourse.mybir as mybir
import concourse.tile as tile

B, DIM, HGT, WID = 16, 512, 64, 64
L = HGT * WID            # 4096
HEADS, DH = 8, 64
HIDDEN = HEADS * DH      # 512
NCORES = 8
BPC = B // NCORES        # 2 batches per core
P = 128
CHUNK = 512
NCHUNK = L // CHUNK      # 8
KT = DIM // P            # 4 contraction tiles over channels
MT = DIM // P            # 4 output row tiles
LM = CHUNK // P          # 4 l-subtiles per chunk
NPAIR = HEADS // 2       # 4 head pairs
VW = DH + 2              # per-head vT width: 64 v cols + 2 ones cols

F32 = mybir.dt.float32
F16 = mybir.dt.float16


def build_kernel(ctx: ExitStack, tc: "tile.TileContext", x_in, wkvT_in, wq_in,
                 woT_in, bias_in, y_out):
    nc = tc.nc

    wpool = ctx.enter_context(tc.tile_pool(name="weights", bufs=1))
    xpool = ctx.enter_context(tc.tile_pool(name="xres", bufs=1))
    epool = ctx.enter_context(tc.tile_pool(name="ev", bufs=6))
    ypool = ctx.enter_context(tc.tile_pool(name="ysb", bufs=1))
    cpool = ctx.enter_context(tc.tile_pool(name="ctxacc", bufs=1))
    fpool = ctx.enter_context(tc.tile_pool(name="fold", bufs=1))
    wypool = ctx.enter_context(tc.tile_pool(name="wyt", bufs=1))
    psk = ctx.enter_context(tc.tile_pool(name="psk", bufs=2, space="PSUM"))
    psv = ctx.enter_context(tc.tile_pool(name="psv", bufs=2, space="PSUM"))
    psy = ctx.enter_context(tc.tile_pool(name="psy", bufs=2, space="PSUM"))
    psc = ctx.enter_context(tc.tile_pool(name="psc", bufs=2, space="PSUM"))

    # ---- input loads. Queue = issuing engine; the Scalar queue also runs
    # the exp activations, so it gets only a bounded number of early
    # dispatches (DMA dispatch instructions block the engine queue on ring
    # flow control). x(b1) is emitted mid-pass-A so its dispatches queue
    # behind batch 0's critical work, not ahead of it. ----
    wk_sb, wv_sb = [], []
    for k in range(KT):
        wk_sb.append(wpool.tile([P, HIDDEN], F16, tag=f"wk{k}",
                                name=f"wk{k}"))
        wv_sb.append(wpool.tile([P, HIDDEN], F16, tag=f"wv{k}",
                                name=f"wv{k}"))
    PAIRW = 2 * CHUNK
    xj_sb = {}  # (b, kt, j) -> [128, 1024] fp16, j = chunk pair
    for b in range(BPC):
        for j in range(NCHUNK // 2):
            for k in range(KT):
                xj_sb[(b, k, j)] = xpool.tile(
                    [P, PAIRW], F16, tag=f"x{b}_{k}_{j}", name=f"x{b}_{k}_{j}")

    def dma_w(eng, t, src_ap):
        eng.dma_start(t[:], src_ap)

    def dma_x(eng, b, k, j):
        eng.dma_start(xj_sb[(b, k, j)][:],
                      x_in[b, k * P:(k + 1) * P,
                           j * PAIRW:(j + 1) * PAIRW])

    # startup-critical loads, explicitly laid out per queue
    dma_w(nc.scalar, wk_sb[0], wkvT_in[0:P, 0:HIDDEN])
    dma_w(nc.scalar, wk_sb[3], wkvT_in[3 * P:4 * P, 0:HIDDEN])
    dma_x(nc.scalar, 0, 1, 0)          # scalar stops here: exp comes next
    dma_x(nc.sync, 0, 0, 0)
    dma_x(nc.sync, 0, 3, 0)
    dma_w(nc.gpsimd, wk_sb[1], wkvT_in[P:2 * P, 0:HIDDEN])
    dma_w(nc.gpsimd, wk_sb[2], wkvT_in[2 * P:3 * P, 0:HIDDEN])
    dma_x(nc.gpsimd, 0, 2, 0)
    dma_w(nc.sync, wv_sb[0], wkvT_in[0:P, HIDDEN:2 * HIDDEN])
    dma_w(nc.gpsimd, wv_sb[1], wkvT_in[P:2 * P, HIDDEN:2 * HIDDEN])
    dma_w(nc.sync, wv_sb[2], wkvT_in[2 * P:3 * P, HIDDEN:2 * HIDDEN])
    dma_w(nc.gpsimd, wv_sb[3], wkvT_in[3 * P:4 * P, HIDDEN:2 * HIDDEN])
    for j in range(1, NCHUNK // 2):
        for k in range(KT):
            dma_x(nc.sync if (k + j) % 2 == 0 else nc.gpsimd, 0, k, j)

    def load_x_batch1():
        for j in range(NCHUNK // 2):
            for k in range(KT):
                dma_x(nc.sync if (k + j) % 2 == 0 else nc.gpsimd, 1, k, j)

    def x_sb(b, k, i):
        return xj_sb[(b, k, i // 2)][:, (i % 2) * CHUNK:(i % 2 + 1) * CHUNK]

    yrow_sb = {}  # (b, u) -> [128, 4096] fp16
    for b in range(BPC):
        for u in range(MT):
            yrow_sb[(b, u)] = ypool.tile([P, L], F16, tag=f"yrow{b}_{u}",
                                         name=f"yrow{b}_{u}")

    # fold-time weights: tiles declared here, DMAs emitted mid-pass-A (they
    # are not needed until fold, and must not clog any queue at startup)
    wq_sb = [wpool.tile([P, DIM], F16, tag=f"wq{p}", name=f"wq{p}")
             for p in range(NPAIR)]
    woT_sb = [wpool.tile([P, DIM], F16, tag=f"wo{p}", name=f"wo{p}")
              for p in range(NPAIR)]
    bias_sb = wpool.tile([P, MT], F32, tag="bias", name="bias")

    def load_fold_weights():
        for p in range(NPAIR):
            nc.sync.dma_start(wq_sb[p][:], wq_in[p * P:(p + 1) * P, :])
            nc.gpsimd.dma_start(woT_sb[p][:], woT_in[p * P:(p + 1) * P, :])
        nc.gpsimd.dma_start(bias_sb[:], bias_in[:])

    wyT_sb = {}  # (b, m) -> [128 c-sub, 512 y] fp16

    def pass_A(b, y_batch=None, trailing_hook=None, ctx_acc=None):
        """kT/vT projections, exp, ctx accumulation for batch b; optionally
        interleaves the y pass of a previous batch. ctx groups of chunk i-1
        are emitted between the kv/y groups of chunk i so their small-N
        matmuls' LDWEIGHTS hide under big matmuls (queue reorder window).
        trailing_hook(p) is called right after pair p's last ctx group so
        per-pair finalize work overlaps the remaining ctx groups."""
        def ctx_group(i, p, E_t, vT_t):
            pc = psc.tile([P, 2 * VW], F32, tag="ctx", name="ctx")
            for lm in range(LM):
                nc.tensor.matmul(
                    pc[:], E_t[lm][:, p * P:(p + 1) * P],
                    vT_t[lm][:, p * 2 * VW:(p + 1) * 2 * VW],
                    start=(lm == 0), stop=(lm == LM - 1))
            if i == 0:
                nc.vector.tensor_copy(ctx_acc[p][:], pc[:])
            else:
                nc.vector.tensor_add(ctx_acc[p][:], ctx_acc[p][:], pc[:])

        prev = None  # (i-1, E_t, vT_t)
        for i in range(NCHUNK):
            E_t, vT_t = [], []
            # all k-groups first: the exp drains chase them with slack, so
            # the next chunk's kps allocation never stalls on ACT
            for lm in range(LM):
                lsl = slice(lm * P, (lm + 1) * P)
                kps = psk.tile([P, CHUNK], F32, tag="kps", name="kps")
                for k in range(KT):
                    nc.tensor.matmul(kps[:], x_sb(b, k, i)[:, lsl],
                                     wk_sb[k][:],
                                     start=(k == 0), stop=(k == KT - 1))
                e = epool.tile([P, CHUNK], F16, tag="E", name="E")
                nc.scalar.activation(e[:], kps[:],
                                     mybir.ActivationFunctionType.Exp)
                E_t.append(e)
            for lm in range(LM):
                lsl = slice(lm * P, (lm + 1) * P)
                vps = psv.tile([P, CHUNK], F32, tag="vps", name="vps")
                for k in range(KT):
                    nc.tensor.matmul(vps[:], x_sb(b, k, i)[:, lsl],
                                     wv_sb[k][:],
                                     start=(k == 0), stop=(k == KT - 1))
                if prev is not None:
                    ctx_group(prev[0], lm, prev[1], prev[2])
                v = epool.tile([P, HEADS * VW], F16, tag="vT", name="vT")
                v_view = v[:].rearrange("p (h e) -> p h e", e=VW)
                nc.vector.tensor_copy(
                    v_view[:, :, 0:DH],
                    vps[:].rearrange("p (h e) -> p h e", e=DH))
                nc.vector.tensor_scalar(
                    v_view[:, :, DH:DH + 2],
                    vps[:].rearrange("p (h e) -> p h e", e=DH)[:, :, 0:2],
                    0.0, 1.0, mybir.AluOpType.mult, mybir.AluOpType.add)
                vT_t.append(v)
            if y_batch is not None:
                # u-strip halves: (u, chunks 0-3) on even i, (u, 4-8) on odd
                u, half = i // 2, i % 2
                y_ublock(y_batch, u, half * (NCHUNK // 2),
                         (half + 1) * (NCHUNK // 2))
            if b == 0 and i == 1:
                load_x_batch1()
            if b == 0 and i == 3:
                load_fold_weights()
            prev = (i, E_t, vT_t)
        for p in range(NPAIR):
            ctx_group(prev[0], p, prev[1], prev[2])
            if trailing_hook is not None:
                trailing_hook(p)
        return ctx_acc

    tmp_sb = {}

    def fold_pair(b, ctx_acc, p):
        """ctx~ = ctx/rowsum; tmp_h = ctx~_h^T @ wq_h (pair p)."""
        acc = ctx_acc[p]
        nc.vector.reciprocal(acc[0:DH, DH:DH + 1], acc[0:DH, DH:DH + 1])
        nc.vector.reciprocal(acc[DH:P, 2 * VW - 2:2 * VW - 1],
                             acc[DH:P, 2 * VW - 2:2 * VW - 1])
        ctxn = fpool.tile([P, DH], F16, tag=f"ctxn{p}", name=f"ctxn{p}")
        nc.vector.tensor_scalar_mul(ctxn[0:DH, :], acc[0:DH, 0:DH],
                                    acc[0:DH, DH:DH + 1])
        nc.vector.tensor_scalar_mul(ctxn[DH:P, :], acc[DH:P, VW:VW + DH],
                                    acc[DH:P, 2 * VW - 2:2 * VW - 1])
        tps = psy.tile([P, CHUNK], F32, tag="yps", name="yps")
        nc.tensor.matmul(tps[0:DH, :], ctxn[0:DH, :], wq_sb[p][0:DH, :],
                         start=True, stop=True, tile_position=(0, 0))
        nc.tensor.matmul(tps[DH:P, :], ctxn[DH:P, :], wq_sb[p][DH:P, :],
                         start=True, stop=True, tile_position=(DH, DH))
        t = fpool.tile([P, CHUNK], F16, tag=f"tmp{p}", name=f"tmp{p}")
        nc.scalar.copy(t[:], tps[:])
        tmp_sb[p] = t

    def fold_wyT(b):
        """WyT[m] = sum_p tmp-pair contraction with w_out^T."""
        for m in range(MT):
            wps = psy.tile([P, CHUNK], F32, tag="yps", name="yps")
            for p in range(NPAIR):
                nc.tensor.matmul(wps[:], tmp_sb[p][:, m * P:(m + 1) * P],
                                 woT_sb[p][:],
                                 start=(p == 0), stop=(p == NPAIR - 1))
            t = wypool.tile([P, DIM], F16, tag=f"wyt{b}_{m}",
                            name=f"wyt{b}_{m}")
            nc.vector.tensor_copy(t[:], wps[:])
            wyT_sb[(b, m)] = t

    def y_ublock(b, u, i0, i1):
        """y chunks [i0, i1) for output row strip u; DMA when the strip is
        complete so the store overlaps the remaining strips' compute."""
        yrow = yrow_sb[(b, u)]
        for i in range(i0, i1):
            ls = slice(i * CHUNK, (i + 1) * CHUNK)
            yps = psy.tile([P, CHUNK], F32, tag="yps", name="yps")
            for k in range(KT):
                nc.tensor.matmul(yps[:],
                                 wyT_sb[(b, k)][:, u * P:(u + 1) * P],
                                 x_sb(b, k, i),
                                 start=(k == 0), stop=(k == KT - 1))
            # drain on ACT (Identity + per-partition bias): keeps DVE, which
            # carries the vT copies and ctx accumulation, off the critical path
            nc.scalar.activation(yrow[:, ls], yps[:],
                                 mybir.ActivationFunctionType.Identity,
                                 bias=bias_sb[:, u:u + 1])
            # fire the store in pieces as chunks complete so only the final
            # 256KB piece is exposed at the end of the kernel
            if i == 3:
                nc.gpsimd.dma_start(
                    y_out[b, u * P:(u + 1) * P, 0:4 * CHUNK],
                    yrow[:, 0:4 * CHUNK])
            elif i == 6:
                nc.gpsimd.dma_start(
                    y_out[b, u * P:(u + 1) * P, 4 * CHUNK:7 * CHUNK],
                    yrow[:, 4 * CHUNK:7 * CHUNK])
            elif i == 7:
                nc.gpsimd.dma_start(
                    y_out[b, u * P:(u + 1) * P, 7 * CHUNK:L],
                    yrow[:, 7 * CHUNK:L])

    def pass_Y(b):
        """y = WyT^T @ x + bias for batch b; DMA out as fp16."""
        for u in range(MT):
            y_ublock(b, u, 0, NCHUNK)

    acc_box = {}

    def hook0(p):
        fold_pair(0, acc_box[0], p)

    def hook1(p):
        fold_pair(1, acc_box[1], p)

    acc_box[0] = [cpool.tile([P, 2 * VW], F32, tag=f"ctxacc{p}",
                             name=f"ctxacc{p}") for p in range(NPAIR)]
    pass_A(0, y_batch=None, trailing_hook=hook0, ctx_acc=acc_box[0])
    fold_wyT(0)
    acc_box[1] = [cpool.tile([P, 2 * VW], F32, tag=f"ctxacc{p}",
                             name=f"ctxacc{p}") for p in range(NPAIR)]
    pass_A(1, y_batch=0, trailing_hook=hook1, ctx_acc=acc_box[1])
    fold_wyT(1)
    pass_Y(1)


def build_module():
    nc = bacc.Bacc("TRN2", target_bir_lowering=False, debug=False,
                   num_devices=NCORES)
    x_in = nc.dram_tensor("x", [BPC, DIM, L], F16, kind="ExternalInput")
    wkvT_in = nc.dram_tensor("w_kvT", [DIM, 2 * HIDDEN], F16,
                             kind="ExternalInput")
    wq_in = nc.dram_tensor("w_q", [HIDDEN, DIM], F16, kind="ExternalInput")
    woT_in = nc.dram_tensor("w_oT", [HIDDEN, DIM], F16, kind="ExternalInput")
    bias_in = nc.dram_tensor("bias", [P, MT], F32, kind="ExternalInput")
    y_out = nc.dram_tensor("y", [BPC, DIM, L], F16, kind="ExternalOutput")
    with tile.TileContext(nc) as tc:
        with ExitStack() as ctx:
            build_kernel(ctx, tc, x_in, wkvT_in, wq_in, woT_in, bias_in,
                         y_out)
    nc.compile()
    return nc


def make_in_maps(x, w_qkv, w_out, b_out):
    x = np.ascontiguousarray(x, dtype=np.float32).reshape(B, DIM, L)
    x16 = x.astype(np.float16)
    w_qkv = np.asarray(w_qkv, dtype=np.float32)
    wq = np.ascontiguousarray(w_qkv[0:HIDDEN]).astype(np.float16)
    wkvT = np.ascontiguousarray(
        np.concatenate([w_qkv[HIDDEN:2 * HIDDEN].T,
                        w_qkv[2 * HIDDEN:3 * HIDDEN].T], axis=1)
    ).astype(np.float16)
    woT = np.ascontiguousarray(
        np.asarray(w_out, dtype=np.float32).T).astype(np.float16)
    bias = np.ascontiguousarray(
        np.asarray(b_out, dtype=np.float32).reshape(MT, P).T)
    in_maps = []
    for c in range(NCORES):
        in_maps.append({
            "x": x16[c * BPC:(c + 1) * BPC],
            "w_kvT": wkvT,
            "w_q": wq,
            "w_oT": woT,
            "bias": bias,
        })
    return in_maps


_NC_CACHE = None


def kernel(x, w_qkv, w_out, b_out, *, trace=False, trace_kwargs=None):
    """Full inputs in, full output out. Shards batch across 8 NeuronCores."""
    global _NC_CACHE
    from concourse.bass_utils import run_bass_kernel_spmd

    if _NC_CACHE is None:
        _NC_CACHE = build_module()
    nc = _NC_CACHE

    in_maps = make_in_maps(x, w_qkv, w_out, b_out)
    kw = dict(trace_kwargs or {})
    res = run_bass_kernel_spmd(nc, in_maps, list(range(NCORES)),
                               trace=trace, **kw)
    y = np.empty((B, DIM, HGT, WID), dtype=np.float32)
    for c in range(NCORES):
        y[c * BPC:(c + 1) * BPC] = res.results[c]["y"].astype(
            np.float32).reshape(BPC, DIM, HGT, WID)
    kernel.last_results = res
    return y


# revision 14
# speedup vs baseline: 1.1000x; 1.0161x over previous
# Trainium2 Bass kernel for LinearAttention — v2 (q-path folded).
#
# Reference computation (per batch element b of 16):
#   qkv = w_qkv @ x[b]; q,k,v split into 8 heads x 64 dims
#   E = exp(k); ctx_h = (E_h/rowsum) @ v_h^T        # [64, 64]
#   y = w_out @ concat(ctx_h^T @ q_h) + b_out
#
# Key algebra: y = Wy @ x + b where
#   Wy = w_out @ blockdiag(ctx~_h^T) @ w_q   (per batch, [512, 512])
# so q never needs to be computed over l. Per batch:
#   Pass A: kT/vT projections (l on partitions), E = exp(kT), ctx
#           accumulation via head-pair matmuls with ones columns in vT
#           producing rowsums.
#   Fold:   ctx~ = ctx/rowsum; tmp_h = ctx~_h^T @ w_q_h;
#           WyT[c, y] = sum_h tmp_h^T-contract-w_outT  ([512, 512])
#   Pass Y: y = WyT^T-contract-x + bias; DMA out (fp16).
#
# All matmuls in fp16 (1 cycle/row at any N; fp32 PSUM accumulate).
# Data-parallel over batch: 16 batches / 8 cores = 2 per core.

import numpy as np
from contextlib import ExitStack

import concourse.bass as bass
import concourse.bacc as bacc
import concourse.mybir as mybir
import concourse.tile as tile

B, DIM, HGT, WID = 16, 512, 64, 64
L = HGT * WID            # 4096
HEADS, DH = 8, 64
HIDDEN = HEADS * DH      # 512
NCORES = 8
BPC = B // NCORES        # 2 batches per core
P = 128
CHUNK = 512
NCHUNK = L // CHUNK      # 8
KT = DIM // P            # 4 contraction tiles over channels
MT = DIM // P            # 4 output row tiles
LM = CHUNK // P          # 4 l-subtiles per chunk
NPAIR = HEADS // 2       # 4 head pairs
VW = DH + 2              # per-head vT width: 64 v cols + 2 ones cols

F32 = mybir.dt.float32
F16 = mybir.dt.float16


def build_kernel(ctx: ExitStack, tc: "tile.TileContext", x_in, wkvT_in, wq_in,
                 woT_in, bias_in, y_out):
    nc = tc.nc

    wpool = ctx.enter_context(tc.tile_pool(name="weights", bufs=1))
    xpool = ctx.enter_context(tc.tile_pool(name="xres", bufs=1))
    epool = ctx.enter_context(tc.tile_pool(name="ev", bufs=6))
    ypool = ctx.enter_context(tc.tile_pool(name="ysb", bufs=1))
    cpool = ctx.enter_context(tc.tile_pool(name="ctxacc", bufs=1))
    fpool = ctx.enter_context(tc.tile_pool(name="fold", bufs=1))
    wypool = ctx.enter_context(tc.tile_pool(name="wyt", bufs=1))
    psk = ctx.enter_context(tc.tile_pool(name="psk", bufs=2, space="PSUM"))
    psv = ctx.enter_context(tc.tile_pool(name="psv", bufs=2, space="PSUM"))
    psy = ctx.enter_context(tc.tile_pool(name="psy", bufs=2, space="PSUM"))
    psc = ctx.enter_context(tc.tile_pool(name="psc", bufs=2, space="PSUM"))

    # ---- input loads. Queue = issuing engine; the Scalar queue also runs
    # the exp activations, so it gets only a bounded number of early
    # dispatches (DMA dispatch instructions block the engine queue on ring
    # flow control). x(b1) is emitted mid-pass-A so its dispatches queue
    # behind batch 0's critical work, not ahead of it. ----
    wk_sb, wv_sb = [], []
    for k in range(KT):
        wk_sb.append(wpool.tile([P, HIDDEN], F16, tag=f"wk{k}",
                                name=f"wk{k}"))
        wv_sb.append(wpool.tile([P, HIDDEN], F16, tag=f"wv{k}",
                                name=f"wv{k}"))
    PAIRW = 2 * CHUNK
    xj_sb = {}  # (b, kt, j) -> [128, 1024] fp16, j = chunk pair
    for b in range(BPC):
        for j in range(NCHUNK // 2):
            for k in range(KT):
                xj_sb[(b, k, j)] = xpool.tile(
                    [P, PAIRW], F16, tag=f"x{b}_{k}_{j}", name=f"x{b}_{k}_{j}")

    def dma_w(eng, t, src_ap):
        eng.dma_start(t[:], src_ap)

    def dma_x(eng, b, k, j):
        eng.dma_start(xj_sb[(b, k, j)][:],
                      x_in[b, k * P:(k + 1) * P,
                           j * PAIRW:(j + 1) * PAIRW])

    # startup-critical loads, explicitly laid out per queue
    dma_w(nc.scalar, wk_sb[0], wkvT_in[0:P, 0:HIDDEN])
    dma_w(nc.scalar, wk_sb[3], wkvT_in[3 * P:4 * P, 0:HIDDEN])
    dma_x(nc.scalar, 0, 1, 0)          # scalar stops here: exp comes next
    dma_x(nc.sync, 0, 0, 0)
    dma_x(nc.sync, 0, 3, 0)
    dma_w(nc.gpsimd, wk_sb[1], wkvT_in[P:2 * P, 0:HIDDEN])
    dma_w(nc.gpsimd, wk_sb[2], wkvT_in[2 * P:3 * P, 0:HIDDEN])
    dma_x(nc.gpsimd, 0, 2, 0)
    dma_w(nc.sync, wv_sb[0], wkvT_in[0:P, HIDDEN:2 * HIDDEN])
    dma_w(nc.gpsimd, wv_sb[1], wkvT_in[P:2 * P, HIDDEN:2 * HIDDEN])
    dma_w(nc.sync, wv_sb[2], wkvT_in[2 * P:3 * P, HIDDEN:2 * HIDDEN])
    dma_w(nc.gpsimd, wv_sb[3], wkvT_in[3 * P:4 * P, HIDDEN:2 * HIDDEN])
    for j in range(1, NCHUNK // 2):
        for k in range(KT):
            dma_x(nc.sync if (k + j) % 2 == 0 else nc.gpsimd, 0, k, j)

    def load_x_batch1():
        for j in range(NCHUNK // 2):
            for k in range(KT):
                dma_x(nc.sync if (k + j) % 2 == 0 else nc.gpsimd, 1, k, j)

    def x_sb(b, k, i):
        return xj_sb[(b, k, i // 2)][:, (i % 2) * CHUNK:(i % 2 + 1) * CHUNK]

    yrow_sb = {}  # (b, u) -> [128, 4096] fp16
    for b in range(BPC):
        for u in range(MT):
            yrow_sb[(b, u)] = ypool.tile([P, L], F16, tag=f"yrow{b}_{u}",
                                         name=f"yrow{b}_{u}")

    # fold-time weights: tiles declared here, DMAs emitted mid-pass-A (they
    # are not needed until fold, and must not clog any queue at startup)
    wq_sb = [wpool.tile([P, DIM], F16, tag=f"wq{p}", name=f"wq{p}")
             for p in range(NPAIR)]
    woT_sb = [wpool.tile([P, DIM], F16, tag=f"wo{p}", name=f"wo{p}")
              for p in range(NPAIR)]
    bias_sb = wpool.tile([P, MT], F32, tag="bias", name="bias")

    def load_fold_weights():
        for p in range(NPAIR):
            nc.sync.dma_start(wq_sb[p][:], wq_in[p * P:(p + 1) * P, :])
            nc.gpsimd.dma_start(woT_sb[p][:], woT_in[p * P:(p + 1) * P, :])
        nc.gpsimd.dma_start(bias_sb[:], bias_in[:])

    wyT_sb = {}  # (b, m) -> [128 c-sub, 512 y] fp16

    def pass_A(b, y_batch=None, trailing_hook=None, ctx_acc=None):
        """kT/vT projections, exp, ctx accumulation for batch b; optionally
        interleaves the y pass of a previous batch. ctx groups of chunk i-1
        are emitted between the kv/y groups of chunk i so their small-N
        matmuls' LDWEIGHTS hide under big matmuls (queue reorder window).
        trailing_hook(p) is called right after pair p's last ctx group so
        per-pair finalize work overlaps the remaining ctx groups."""
        def ctx_group(i, p, E_t, vT_t):
            pc = psc.tile([P, 2 * VW], F32, tag="ctx", name="ctx")
            for lm in range(LM):
                nc.tensor.matmul(
                    pc[:], E_t[lm][:, p * P:(p + 1) * P],
                    vT_t[lm][:, p * 2 * VW:(p + 1) * 2 * VW],
                    start=(lm == 0), stop=(lm == LM - 1))
            if i == 0:
                nc.vector.tensor_copy(ctx_acc[p][:], pc[:])
            else:
                nc.vector.tensor_add(ctx_acc[p][:], ctx_acc[p][:], pc[:])

        prev = None  # (i-1, E_t, vT_t)
        for i in range(NCHUNK):
            E_t, vT_t = [], []
            # all k-groups first: the exp drains chase them with slack, so
            # the next chunk's kps allocation never stalls on ACT
            for lm in range(LM):
                lsl = slice(lm * P, (lm + 1) * P)
                # during batch 0's pass A the y psum pool is idle — alternate
                # with it so the exp drain never gates the next k-group
                kpool = psy if (y_batch is None and lm % 2 == 1) else psk
                ktag = "yps" if kpool is psy else "kps"
                kps = kpool.tile([P, CHUNK], F32, tag=ktag, name=ktag)
                for k in range(KT):
                    nc.tensor.matmul(kps[:], x_sb(b, k, i)[:, lsl],
                                     wk_sb[k][:],
                                     start=(k == 0), stop=(k == KT - 1))
                e = epool.tile([P, CHUNK], F16, tag="E", name="E")
                nc.scalar.activation(e[:], kps[:],
                                     mybir.ActivationFunctionType.Exp)
                E_t.append(e)
            for lm in range(LM):
                lsl = slice(lm * P, (lm + 1) * P)
                vps = psv.tile([P, CHUNK], F32, tag="vps", name="vps")
                for k in range(KT):
                    nc.tensor.matmul(vps[:], x_sb(b, k, i)[:, lsl],
                                     wv_sb[k][:],
                                     start=(k == 0), stop=(k == KT - 1))
                if prev is not None:
                    ctx_group(prev[0], lm, prev[1], prev[2])
                v = epool.tile([P, HEADS * VW], F16, tag="vT", name="vT")
                v_view = v[:].rearrange("p (h e) -> p h e", e=VW)
                nc.vector.tensor_copy(
                    v_view[:, :, 0:DH],
                    vps[:].rearrange("p (h e) -> p h e", e=DH))
                nc.vector.tensor_scalar(
                    v_view[:, :, DH:DH + 2],
                    vps[:].rearrange("p (h e) -> p h e", e=DH)[:, :, 0:2],
                    0.0, 1.0, mybir.AluOpType.mult, mybir.AluOpType.add)
                vT_t.append(v)
            if y_batch is not None:
                # u-strip halves: (u, chunks 0-3) on even i, (u, 4-8) on odd
                u, half = i // 2, i % 2
                y_ublock(y_batch, u, half * (NCHUNK // 2),
                         (half + 1) * (NCHUNK // 2))
            if b == 0 and i == 1:
                load_x_batch1()
            if b == 0 and i == 3:
                load_fold_weights()
            prev = (i, E_t, vT_t)
        for p in range(NPAIR):
            ctx_group(prev[0], p, prev[1], prev[2])
            if trailing_hook is not None:
                trailing_hook(p)
        return ctx_acc

    tmp_sb = {}

    def fold_pair(b, ctx_acc, p):
        """ctx~ = ctx/rowsum; tmp_h = ctx~_h^T @ wq_h (pair p)."""
        acc = ctx_acc[p]
        nc.vector.reciprocal(acc[0:DH, DH:DH + 1], acc[0:DH, DH:DH + 1])
        nc.vector.reciprocal(acc[DH:P, 2 * VW - 2:2 * VW - 1],
                             acc[DH:P, 2 * VW - 2:2 * VW - 1])
        ctxn = fpool.tile([P, DH], F16, tag=f"ctxn{p}", name=f"ctxn{p}")
        nc.vector.tensor_scalar_mul(ctxn[0:DH, :], acc[0:DH, 0:DH],
                                    acc[0:DH, DH:DH + 1])
        nc.vector.tensor_scalar_mul(ctxn[DH:P, :], acc[DH:P, VW:VW + DH],
                                    acc[DH:P, 2 * VW - 2:2 * VW - 1])
        tps = psy.tile([P, CHUNK], F32, tag="yps", name="yps")
        nc.tensor.matmul(tps[0:DH, :], ctxn[0:DH, :], wq_sb[p][0:DH, :],
                         start=True, stop=True, tile_position=(0, 0))
        nc.tensor.matmul(tps[DH:P, :], ctxn[DH:P, :], wq_sb[p][DH:P, :],
                         start=True, stop=True, tile_position=(DH, DH))
        t = fpool.tile([P, CHUNK], F16, tag=f"tmp{p}", name=f"tmp{p}")
        nc.scalar.copy(t[:], tps[:])
        tmp_sb[p] = t

    def fold_wyT(b):
        """WyT[m] = sum_p tmp-pair contraction with w_out^T."""
        for m in range(MT):
            wps = psy.tile([P, CHUNK], F32, tag="yps", name="yps")
            for p in range(NPAIR):
                nc.tensor.matmul(wps[:], tmp_sb[p][:, m * P:(m + 1) * P],
                                 woT_sb[p][:],
                                 start=(p == 0), stop=(p == NPAIR - 1))
            t = wypool.tile([P, DIM], F16, tag=f"wyt{b}_{m}",
                            name=f"wyt{b}_{m}")
            nc.vector.tensor_copy(t[:], wps[:])
            wyT_sb[(b, m)] = t

    def y_ublock(b, u, i0, i1):
        """y chunks [i0, i1) for output row strip u; DMA when the strip is
        complete so the store overlaps the remaining strips' compute."""
        yrow = yrow_sb[(b, u)]
        for i in range(i0, i1):
            ls = slice(i * CHUNK, (i + 1) * CHUNK)
            yps = psy.tile([P, CHUNK], F32, tag="yps", name="yps")
            for k in range(KT):
                nc.tensor.matmul(yps[:],
                                 wyT_sb[(b, k)][:, u * P:(u + 1) * P],
                                 x_sb(b, k, i),
                                 start=(k == 0), stop=(k == KT - 1))
            # drain on ACT (Identity + per-partition bias): keeps DVE, which
            # carries the vT copies and ctx accumulation, off the critical path
            nc.scalar.activation(yrow[:, ls], yps[:],
                                 mybir.ActivationFunctionType.Identity,
                                 bias=bias_sb[:, u:u + 1])
            # fire the store in pieces as chunks complete so only the final
            # 256KB piece is exposed at the end of the kernel
            if i == 3:
                nc.gpsimd.dma_start(
                    y_out[b, u * P:(u + 1) * P, 0:4 * CHUNK],
                    yrow[:, 0:4 * CHUNK])
            elif i == 6:
                nc.gpsimd.dma_start(
                    y_out[b, u * P:(u + 1) * P, 4 * CHUNK:7 * CHUNK],
                    yrow[:, 4 * CHUNK:7 * CHUNK])
            elif i == 7:
                nc.gpsimd.dma_start(
                    y_out[b, u * P:(u + 1) * P, 7 * CHUNK:L],
                    yrow[:, 7 * CHUNK:L])

    def pass_Y(b):
        """y = WyT^T @ x + bias for batch b; DMA out as fp16."""
        for u in range(MT):
            y_ublock(b, u, 0, NCHUNK)

    acc_box = {}

    def hook0(p):
        fold_pair(0, acc_box[0], p)

    def hook1(p):
        fold_pair(1, acc_box[1], p)

    acc_box[0] = [cpool.tile([P, 2 * VW], F32, tag=f"ctxacc{p}",
                             name=f"ctxacc{p}") for p in range(NPAIR)]
    pass_A(0, y_batch=None, trailing_hook=hook0, ctx_acc=acc_box[0])
    fold_wyT(0)
    acc_box[1] = [cpool.tile([P, 2 * VW], F32, tag=f"ctxacc{p}",
                             name=f"ctxacc{p}") for p in range(NPAIR)]
    pass_A(1, y_batch=0, trailing_hook=hook1, ctx_acc=acc_box[1])
    fold_wyT(1)
    pass_Y(1)


def build_module():
    nc = bacc.Bacc("TRN2", target_bir_lowering=False, debug=False,
                   num_devices=NCORES)
    x_in = nc.dram_tensor("x", [BPC, DIM, L], F16, kind="ExternalInput")
    wkvT_in = nc.dram_tensor("w_kvT", [DIM, 2 * HIDDEN], F16,
                             kind="ExternalInput")
    wq_in = nc.dram_tensor("w_q", [HIDDEN, DIM], F16, kind="ExternalInput")
    woT_in = nc.dram_tensor("w_oT", [HIDDEN, DIM], F16, kind="ExternalInput")
    bias_in = nc.dram_tensor("bias", [P, MT], F32, kind="ExternalInput")
    y_out = nc.dram_tensor("y", [BPC, DIM, L], F16, kind="ExternalOutput")
    with tile.TileContext(nc) as tc:
        with ExitStack() as ctx:
            build_kernel(ctx, tc, x_in, wkvT_in, wq_in, woT_in, bias_in,
                         y_out)
    nc.compile()
    return nc


def make_in_maps(x, w_qkv, w_out, b_out):
    x = np.ascontiguousarray(x, dtype=np.float32).reshape(B, DIM, L)
    x16 = x.astype(np.float16)
    w_qkv = np.asarray(w_qkv, dtype=np.float32)
    wq = np.ascontiguousarray(w_qkv[0:HIDDEN]).astype(np.float16)
    wkvT = np.ascontiguousarray(
        np.concatenate([w_qkv[HIDDEN:2 * HIDDEN].T,
                        w_qkv[2 * HIDDEN:3 * HIDDEN].T], axis=1)
    ).astype(np.float16)
    woT = np.ascontiguousarray(
        np.asarray(w_out, dtype=np.float32).T).astype(np.float16)
    bias = np.ascontiguousarray(
        np.asarray(b_out, dtype=np.float32).reshape(MT, P).T)
    in_maps = []
    for c in range(NCORES):
        in_maps.append({
            "x": x16[c * BPC:(c + 1) * BPC],
            "w_kvT": wkvT,
            "w_q": wq,
            "w_oT": woT,
            "bias": bias,
        })
    return in_maps


_NC_CACHE = None


def kernel(x, w_qkv, w_out, b_out, *, trace=False, trace_kwargs=None):
    """Full inputs in, full output out. Shards batch across 8 NeuronCores."""
    global _NC_CACHE
    from concourse.bass_utils import run_bass_kernel_spmd

    if _NC_CACHE is None:
        _NC_CACHE = build_module()
    nc = _NC_CACHE

    in_maps = make_in_maps(x, w_qkv, w_out, b_out)
    kw = dict(trace_kwargs or {})
    res = run_bass_kernel_spmd(nc, in_maps, list(range(NCORES)),
                               trace=trace, **kw)
    y = np.empty((B, DIM, HGT, WID), dtype=np.float32)
    for c in range(NCORES):
        y[c * BPC:(c + 1) * BPC] = res.results[c]["y"].astype(
            np.float32).reshape(BPC, DIM, HGT, WID)
    kernel.last_results = res
    return y


# revision 15
# speedup vs baseline: 1.1032x; 1.0029x over previous
# Trainium2 Bass kernel for LinearAttention — v2 (q-path folded).
#
# Reference computation (per batch element b of 16):
#   qkv = w_qkv @ x[b]; q,k,v split into 8 heads x 64 dims
#   E = exp(k); ctx_h = (E_h/rowsum) @ v_h^T        # [64, 64]
#   y = w_out @ concat(ctx_h^T @ q_h) + b_out
#
# Key algebra: y = Wy @ x + b where
#   Wy = w_out @ blockdiag(ctx~_h^T) @ w_q   (per batch, [512, 512])
# so q never needs to be computed over l. Per batch:
#   Pass A: kT/vT projections (l on partitions), E = exp(kT), ctx
#           accumulation via head-pair matmuls with ones columns in vT
#           producing rowsums.
#   Fold:   ctx~ = ctx/rowsum; tmp_h = ctx~_h^T @ w_q_h;
#           WyT[c, y] = sum_h tmp_h^T-contract-w_outT  ([512, 512])
#   Pass Y: y = WyT^T-contract-x + bias; DMA out (fp16).
#
# All matmuls in fp16 (1 cycle/row at any N; fp32 PSUM accumulate).
# Data-parallel over batch: 16 batches / 8 cores = 2 per core.

import numpy as np
from contextlib import ExitStack

import concourse.bass as bass
import concourse.bacc as bacc
import concourse.mybir as mybir
import concourse.tile as tile

B, DIM, HGT, WID = 16, 512, 64, 64
L = HGT * WID            # 4096
HEADS, DH = 8, 64
HIDDEN = HEADS * DH      # 512
NCORES = 8
BPC = B // NCORES        # 2 batches per core
P = 128
CHUNK = 512
NCHUNK = L // CHUNK      # 8
KT = DIM // P            # 4 contraction tiles over channels
MT = DIM // P            # 4 output row tiles
LM = CHUNK // P          # 4 l-subtiles per chunk
NPAIR = HEADS // 2       # 4 head pairs
VW = DH + 2              # per-head vT width: 64 v cols + 2 ones cols

F32 = mybir.dt.float32
F16 = mybir.dt.float16


def build_kernel(ctx: ExitStack, tc: "tile.TileContext", x_in, wkvT_in, wq_in,
                 woT_in, bias_in, y_out):
    nc = tc.nc

    wpool = ctx.enter_context(tc.tile_pool(name="weights", bufs=1))
    xpool = ctx.enter_context(tc.tile_pool(name="xres", bufs=1))
    epool = ctx.enter_context(tc.tile_pool(name="ev", bufs=6))
    ypool = ctx.enter_context(tc.tile_pool(name="ysb", bufs=1))
    cpool = ctx.enter_context(tc.tile_pool(name="ctxacc", bufs=1))
    fpool = ctx.enter_context(tc.tile_pool(name="fold", bufs=1))
    wypool = ctx.enter_context(tc.tile_pool(name="wyt", bufs=1))
    psk = ctx.enter_context(tc.tile_pool(name="psk", bufs=2, space="PSUM"))
    psv = ctx.enter_context(tc.tile_pool(name="psv", bufs=2, space="PSUM"))
    psy = ctx.enter_context(tc.tile_pool(name="psy", bufs=2, space="PSUM"))
    psc = ctx.enter_context(tc.tile_pool(name="psc", bufs=2, space="PSUM"))

    # ---- input loads. Queue = issuing engine; the Scalar queue also runs
    # the exp activations, so it gets only a bounded number of early
    # dispatches (DMA dispatch instructions block the engine queue on ring
    # flow control). x(b1) is emitted mid-pass-A so its dispatches queue
    # behind batch 0's critical work, not ahead of it. ----
    wk_sb, wv_sb = [], []
    for k in range(KT):
        wk_sb.append(wpool.tile([P, HIDDEN], F16, tag=f"wk{k}",
                                name=f"wk{k}"))
        wv_sb.append(wpool.tile([P, HIDDEN], F16, tag=f"wv{k}",
                                name=f"wv{k}"))
    PAIRW = 2 * CHUNK
    xj_sb = {}  # (b, kt, j) -> [128, 1024] fp16, j = chunk pair
    for b in range(BPC):
        for j in range(NCHUNK // 2):
            for k in range(KT):
                xj_sb[(b, k, j)] = xpool.tile(
                    [P, PAIRW], F16, tag=f"x{b}_{k}_{j}", name=f"x{b}_{k}_{j}")

    def dma_w(eng, t, src_ap):
        eng.dma_start(t[:], src_ap)

    def dma_x(eng, b, k, j):
        eng.dma_start(xj_sb[(b, k, j)][:],
                      x_in[b, k * P:(k + 1) * P,
                           j * PAIRW:(j + 1) * PAIRW])

    # startup-critical loads, laid out so each k-group operand (wk[k] +
    # x pair-0[k]) lands just in time for its matmul in the first chunk
    dma_w(nc.scalar, wk_sb[0], wkvT_in[0:P, 0:HIDDEN])
    dma_w(nc.scalar, wk_sb[3], wkvT_in[3 * P:4 * P, 0:HIDDEN])
    dma_x(nc.scalar, 0, 1, 0)          # scalar stops here: exp comes next
    dma_x(nc.sync, 0, 0, 0)
    dma_w(nc.sync, wk_sb[1], wkvT_in[P:2 * P, 0:HIDDEN])
    dma_x(nc.sync, 0, 3, 0)
    dma_x(nc.gpsimd, 0, 2, 0)
    dma_w(nc.gpsimd, wk_sb[2], wkvT_in[2 * P:3 * P, 0:HIDDEN])
    dma_w(nc.gpsimd, wv_sb[1], wkvT_in[P:2 * P, HIDDEN:2 * HIDDEN])
    dma_w(nc.sync, wv_sb[0], wkvT_in[0:P, HIDDEN:2 * HIDDEN])
    dma_w(nc.gpsimd, wv_sb[3], wkvT_in[3 * P:4 * P, HIDDEN:2 * HIDDEN])
    dma_w(nc.sync, wv_sb[2], wkvT_in[2 * P:3 * P, HIDDEN:2 * HIDDEN])
    for j in range(1, NCHUNK // 2):
        for k in range(KT):
            dma_x(nc.sync if (k + j) % 2 == 0 else nc.gpsimd, 0, k, j)

    def load_x_batch1():
        for j in range(NCHUNK // 2):
            for k in range(KT):
                dma_x(nc.sync if (k + j) % 2 == 0 else nc.gpsimd, 1, k, j)

    def x_sb(b, k, i):
        return xj_sb[(b, k, i // 2)][:, (i % 2) * CHUNK:(i % 2 + 1) * CHUNK]

    yrow_sb = {}  # (b, u) -> [128, 4096] fp16
    for b in range(BPC):
        for u in range(MT):
            yrow_sb[(b, u)] = ypool.tile([P, L], F16, tag=f"yrow{b}_{u}",
                                         name=f"yrow{b}_{u}")

    # fold-time weights: tiles declared here, DMAs emitted mid-pass-A (they
    # are not needed until fold, and must not clog any queue at startup)
    wq_sb = [wpool.tile([P, DIM], F16, tag=f"wq{p}", name=f"wq{p}")
             for p in range(NPAIR)]
    woT_sb = [wpool.tile([P, DIM], F16, tag=f"wo{p}", name=f"wo{p}")
              for p in range(NPAIR)]
    bias_sb = wpool.tile([P, MT], F32, tag="bias", name="bias")

    def load_fold_weights():
        for p in range(NPAIR):
            nc.sync.dma_start(wq_sb[p][:], wq_in[p * P:(p + 1) * P, :])
            nc.gpsimd.dma_start(woT_sb[p][:], woT_in[p * P:(p + 1) * P, :])
        nc.gpsimd.dma_start(bias_sb[:], bias_in[:])

    wyT_sb = {}  # (b, m) -> [128 c-sub, 512 y] fp16

    def pass_A(b, y_batch=None, trailing_hook=None, ctx_acc=None):
        """kT/vT projections, exp, ctx accumulation for batch b; optionally
        interleaves the y pass of a previous batch. ctx groups of chunk i-1
        are emitted between the kv/y groups of chunk i so their small-N
        matmuls' LDWEIGHTS hide under big matmuls (queue reorder window).
        trailing_hook(p) is called right after pair p's last ctx group so
        per-pair finalize work overlaps the remaining ctx groups."""
        def ctx_group(i, p, E_t, vT_t):
            pc = psc.tile([P, 2 * VW], F32, tag="ctx", name="ctx")
            for lm in range(LM):
                nc.tensor.matmul(
                    pc[:], E_t[lm][:, p * P:(p + 1) * P],
                    vT_t[lm][:, p * 2 * VW:(p + 1) * 2 * VW],
                    start=(lm == 0), stop=(lm == LM - 1))
            if i == 0:
                nc.vector.tensor_copy(ctx_acc[p][:], pc[:])
            else:
                nc.vector.tensor_add(ctx_acc[p][:], ctx_acc[p][:], pc[:])

        prev = None  # (i-1, E_t, vT_t)
        for i in range(NCHUNK):
            E_t, vT_t = [], []
            # all k-groups first: the exp drains chase them with slack, so
            # the next chunk's kps allocation never stalls on ACT
            for lm in range(LM):
                lsl = slice(lm * P, (lm + 1) * P)
                # during batch 0's pass A the y psum pool is idle — alternate
                # with it so the exp drain never gates the next k-group
                kpool = psy if (y_batch is None and lm % 2 == 1) else psk
                ktag = "yps" if kpool is psy else "kps"
                kps = kpool.tile([P, CHUNK], F32, tag=ktag, name=ktag)
                for k in range(KT):
                    nc.tensor.matmul(kps[:], x_sb(b, k, i)[:, lsl],
                                     wk_sb[k][:],
                                     start=(k == 0), stop=(k == KT - 1))
                e = epool.tile([P, CHUNK], F16, tag="E", name="E")
                nc.scalar.activation(e[:], kps[:],
                                     mybir.ActivationFunctionType.Exp)
                E_t.append(e)
            for lm in range(LM):
                lsl = slice(lm * P, (lm + 1) * P)
                vps = psv.tile([P, CHUNK], F32, tag="vps", name="vps")
                for k in range(KT):
                    nc.tensor.matmul(vps[:], x_sb(b, k, i)[:, lsl],
                                     wv_sb[k][:],
                                     start=(k == 0), stop=(k == KT - 1))
                if prev is not None:
                    ctx_group(prev[0], lm, prev[1], prev[2])
                v = epool.tile([P, HEADS * VW], F16, tag="vT", name="vT")
                v_view = v[:].rearrange("p (h e) -> p h e", e=VW)
                nc.vector.tensor_copy(
                    v_view[:, :, 0:DH],
                    vps[:].rearrange("p (h e) -> p h e", e=DH))
                nc.vector.tensor_scalar(
                    v_view[:, :, DH:DH + 2],
                    vps[:].rearrange("p (h e) -> p h e", e=DH)[:, :, 0:2],
                    0.0, 1.0, mybir.AluOpType.mult, mybir.AluOpType.add)
                vT_t.append(v)
            if y_batch is not None:
                # u-strip halves: (u, chunks 0-3) on even i, (u, 4-8) on odd
                u, half = i // 2, i % 2
                y_ublock(y_batch, u, half * (NCHUNK // 2),
                         (half + 1) * (NCHUNK // 2))
            if b == 0 and i == 1:
                load_x_batch1()
            if b == 0 and i == 3:
                load_fold_weights()
            prev = (i, E_t, vT_t)
        for p in range(NPAIR):
            ctx_group(prev[0], p, prev[1], prev[2])
            if trailing_hook is not None:
                trailing_hook(p)
        return ctx_acc

    tmp_sb = {}

    def fold_pair(b, ctx_acc, p):
        """ctx~ = ctx/rowsum; tmp_h = ctx~_h^T @ wq_h (pair p)."""
        acc = ctx_acc[p]
        nc.vector.reciprocal(acc[0:DH, DH:DH + 1], acc[0:DH, DH:DH + 1])
        nc.vector.reciprocal(acc[DH:P, 2 * VW - 2:2 * VW - 1],
                             acc[DH:P, 2 * VW - 2:2 * VW - 1])
        ctxn = fpool.tile([P, DH], F16, tag=f"ctxn{p}", name=f"ctxn{p}")
        nc.vector.tensor_scalar_mul(ctxn[0:DH, :], acc[0:DH, 0:DH],
                                    acc[0:DH, DH:DH + 1])
        nc.vector.tensor_scalar_mul(ctxn[DH:P, :], acc[DH:P, VW:VW + DH],
                                    acc[DH:P, 2 * VW - 2:2 * VW - 1])
        tps = psy.tile([P, CHUNK], F32, tag="yps", name="yps")
        nc.tensor.matmul(tps[0:DH, :], ctxn[0:DH, :], wq_sb[p][0:DH, :],
                         start=True, stop=True, tile_position=(0, 0))
        nc.tensor.matmul(tps[DH:P, :], ctxn[DH:P, :], wq_sb[p][DH:P, :],
                         start=True, stop=True, tile_position=(DH, DH))
        t = fpool.tile([P, CHUNK], F16, tag=f"tmp{p}", name=f"tmp{p}")
        nc.scalar.copy(t[:], tps[:])
        tmp_sb[p] = t

    def fold_wyT(b):
        """WyT[m] = sum_p tmp-pair contraction with w_out^T."""
        for m in range(MT):
            wps = psy.tile([P, CHUNK], F32, tag="yps", name="yps")
            for p in range(NPAIR):
                nc.tensor.matmul(wps[:], tmp_sb[p][:, m * P:(m + 1) * P],
                                 woT_sb[p][:],
                                 start=(p == 0), stop=(p == NPAIR - 1))
            t = wypool.tile([P, DIM], F16, tag=f"wyt{b}_{m}",
                            name=f"wyt{b}_{m}")
            nc.vector.tensor_copy(t[:], wps[:])
            wyT_sb[(b, m)] = t

    def y_ublock(b, u, i0, i1):
        """y chunks [i0, i1) for output row strip u; DMA when the strip is
        complete so the store overlaps the remaining strips' compute."""
        yrow = yrow_sb[(b, u)]
        for i in range(i0, i1):
            ls = slice(i * CHUNK, (i + 1) * CHUNK)
            yps = psy.tile([P, CHUNK], F32, tag="yps", name="yps")
            for k in range(KT):
                nc.tensor.matmul(yps[:],
                                 wyT_sb[(b, k)][:, u * P:(u + 1) * P],
                                 x_sb(b, k, i),
                                 start=(k == 0), stop=(k == KT - 1))
            # drain on ACT (Identity + per-partition bias): keeps DVE, which
            # carries the vT copies and ctx accumulation, off the critical path
            nc.scalar.activation(yrow[:, ls], yps[:],
                                 mybir.ActivationFunctionType.Identity,
                                 bias=bias_sb[:, u:u + 1])
            # fire the store in pieces as chunks complete so only the final
            # 256KB piece is exposed at the end of the kernel
            if i == 3:
                nc.gpsimd.dma_start(
                    y_out[b, u * P:(u + 1) * P, 0:4 * CHUNK],
                    yrow[:, 0:4 * CHUNK])
            elif i == 6:
                nc.gpsimd.dma_start(
                    y_out[b, u * P:(u + 1) * P, 4 * CHUNK:7 * CHUNK],
                    yrow[:, 4 * CHUNK:7 * CHUNK])
            elif i == 7:
                nc.gpsimd.dma_start(
                    y_out[b, u * P:(u + 1) * P, 7 * CHUNK:L],
                    yrow[:, 7 * CHUNK:L])

    def pass_Y(b):
        """y = WyT^T @ x + bias for batch b; DMA out as fp16."""
        for u in range(MT):
            y_ublock(b, u, 0, NCHUNK)

    acc_box = {}

    def hook0(p):
        fold_pair(0, acc_box[0], p)

    def hook1(p):
        fold_pair(1, acc_box[1], p)

    acc_box[0] = [cpool.tile([P, 2 * VW], F32, tag=f"ctxacc{p}",
                             name=f"ctxacc{p}") for p in range(NPAIR)]
    pass_A(0, y_batch=None, trailing_hook=hook0, ctx_acc=acc_box[0])
    fold_wyT(0)
    acc_box[1] = [cpool.tile([P, 2 * VW], F32, tag=f"ctxacc{p}",
                             name=f"ctxacc{p}") for p in range(NPAIR)]
    pass_A(1, y_batch=0, trailing_hook=hook1, ctx_acc=acc_box[1])
    fold_wyT(1)
    pass_Y(1)


def build_module():
    nc = bacc.Bacc("TRN2", target_bir_lowering=False, debug=False,
                   num_devices=NCORES)
    x_in = nc.dram_tensor("x", [BPC, DIM, L], F16, kind="ExternalInput")
    wkvT_in = nc.dram_tensor("w_kvT", [DIM, 2 * HIDDEN], F16,
                             kind="ExternalInput")
    wq_in = nc.dram_tensor("w_q", [HIDDEN, DIM], F16, kind="ExternalInput")
    woT_in = nc.dram_tensor("w_oT", [HIDDEN, DIM], F16, kind="ExternalInput")
    bias_in = nc.dram_tensor("bias", [P, MT], F32, kind="ExternalInput")
    y_out = nc.dram_tensor("y", [BPC, DIM, L], F16, kind="ExternalOutput")
    with tile.TileContext(nc) as tc:
        with ExitStack() as ctx:
            build_kernel(ctx, tc, x_in, wkvT_in, wq_in, woT_in, bias_in,
                         y_out)
    nc.compile()
    return nc


def make_in_maps(x, w_qkv, w_out, b_out):
    x = np.ascontiguousarray(x, dtype=np.float32).reshape(B, DIM, L)
    x16 = x.astype(np.float16)
    w_qkv = np.asarray(w_qkv, dtype=np.float32)
    wq = np.ascontiguousarray(w_qkv[0:HIDDEN]).astype(np.float16)
    wkvT = np.ascontiguousarray(
        np.concatenate([w_qkv[HIDDEN:2 * HIDDEN].T,
                        w_qkv[2 * HIDDEN:3 * HIDDEN].T], axis=1)
    ).astype(np.float16)
    woT = np.ascontiguousarray(
        np.asarray(w_out, dtype=np.float32).T).astype(np.float16)
    bias = np.ascontiguousarray(
        np.asarray(b_out, dtype=np.float32).reshape(MT, P).T)
    in_maps = []
    for c in range(NCORES):
        in_maps.append({
            "x": x16[c * BPC:(c + 1) * BPC],
            "w_kvT": wkvT,
            "w_q": wq,
            "w_oT": woT,
            "bias": bias,
        })
    return in_maps


_NC_CACHE = None


def kernel(x, w_qkv, w_out, b_out, *, trace=False, trace_kwargs=None):
    """Full inputs in, full output out. Shards batch across 8 NeuronCores."""
    global _NC_CACHE
    from concourse.bass_utils import run_bass_kernel_spmd

    if _NC_CACHE is None:
        _NC_CACHE = build_module()
    nc = _NC_CACHE

    in_maps = make_in_maps(x, w_qkv, w_out, b_out)
    kw = dict(trace_kwargs or {})
    res = run_bass_kernel_spmd(nc, in_maps, list(range(NCORES)),
                               trace=trace, **kw)
    y = np.empty((B, DIM, HGT, WID), dtype=np.float32)
    for c in range(NCORES):
        y[c * BPC:(c + 1) * BPC] = res.results[c]["y"].astype(
            np.float32).reshape(BPC, DIM, HGT, WID)
    kernel.last_results = res
    return y


# revision 16
# speedup vs baseline: 1.1057x; 1.0023x over previous
# Trainium2 Bass kernel for LinearAttention — v2 (q-path folded).
#
# Reference computation (per batch element b of 16):
#   qkv = w_qkv @ x[b]; q,k,v split into 8 heads x 64 dims
#   E = exp(k); ctx_h = (E_h/rowsum) @ v_h^T        # [64, 64]
#   y = w_out @ concat(ctx_h^T @ q_h) + b_out
#
# Key algebra: y = Wy @ x + b where
#   Wy = w_out @ blockdiag(ctx~_h^T) @ w_q   (per batch, [512, 512])
# so q never needs to be computed over l. Per batch:
#   Pass A: kT/vT projections (l on partitions), E = exp(kT), ctx
#           accumulation via head-pair matmuls with ones columns in vT
#           producing rowsums.
#   Fold:   ctx~ = ctx/rowsum; tmp_h = ctx~_h^T @ w_q_h;
#           WyT[c, y] = sum_h tmp_h^T-contract-w_outT  ([512, 512])
#   Pass Y: y = WyT^T-contract-x + bias; DMA out (fp16).
#
# All matmuls in fp16 (1 cycle/row at any N; fp32 PSUM accumulate).
# Data-parallel over batch: 16 batches / 8 cores = 2 per core.

import numpy as np
from contextlib import ExitStack

import concourse.bass as bass
import concourse.bacc as bacc
import concourse.mybir as mybir
import concourse.tile as tile

B, DIM, HGT, WID = 16, 512, 64, 64
L = HGT * WID            # 4096
HEADS, DH = 8, 64
HIDDEN = HEADS * DH      # 512
NCORES = 8
BPC = B // NCORES        # 2 batches per core
P = 128
CHUNK = 512
NCHUNK = L // CHUNK      # 8
KT = DIM // P            # 4 contraction tiles over channels
MT = DIM // P            # 4 output row tiles
LM = CHUNK // P          # 4 l-subtiles per chunk
NPAIR = HEADS // 2       # 4 head pairs
VW = DH + 2              # per-head vT width: 64 v cols + 2 ones cols

F32 = mybir.dt.float32
F16 = mybir.dt.float16


def build_kernel(ctx: ExitStack, tc: "tile.TileContext", x_in, wkvT_in, wq_in,
                 woT_in, bias_in, y_out):
    nc = tc.nc

    wpool = ctx.enter_context(tc.tile_pool(name="weights", bufs=1))
    xpool = ctx.enter_context(tc.tile_pool(name="xres", bufs=1))
    epool = ctx.enter_context(tc.tile_pool(name="ev", bufs=6))
    ypool = ctx.enter_context(tc.tile_pool(name="ysb", bufs=1))
    cpool = ctx.enter_context(tc.tile_pool(name="ctxacc", bufs=1))
    fpool = ctx.enter_context(tc.tile_pool(name="fold", bufs=1))
    wypool = ctx.enter_context(tc.tile_pool(name="wyt", bufs=1))
    psk = ctx.enter_context(tc.tile_pool(name="psk", bufs=2, space="PSUM"))
    psv = ctx.enter_context(tc.tile_pool(name="psv", bufs=2, space="PSUM"))
    psy = ctx.enter_context(tc.tile_pool(name="psy", bufs=2, space="PSUM"))
    psc = ctx.enter_context(tc.tile_pool(name="psc", bufs=2, space="PSUM"))

    # ---- input loads. Queue = issuing engine; the Scalar queue also runs
    # the exp activations, so it gets only a bounded number of early
    # dispatches (DMA dispatch instructions block the engine queue on ring
    # flow control). x(b1) is emitted mid-pass-A so its dispatches queue
    # behind batch 0's critical work, not ahead of it. ----
    wk_sb, wv_sb = [], []
    for k in range(KT):
        wk_sb.append(wpool.tile([P, HIDDEN], F16, tag=f"wk{k}",
                                name=f"wk{k}"))
        wv_sb.append(wpool.tile([P, HIDDEN], F16, tag=f"wv{k}",
                                name=f"wv{k}"))
    PAIRW = 2 * CHUNK
    xj_sb = {}  # (b, kt, j) -> [128, 1024] fp16, j = chunk pair
    for b in range(BPC):
        for j in range(NCHUNK // 2):
            for k in range(KT):
                xj_sb[(b, k, j)] = xpool.tile(
                    [P, PAIRW], F16, tag=f"x{b}_{k}_{j}", name=f"x{b}_{k}_{j}")

    def dma_w(eng, t, src_ap):
        eng.dma_start(t[:], src_ap)

    def dma_x(eng, b, k, j):
        eng.dma_start(xj_sb[(b, k, j)][:],
                      x_in[b, k * P:(k + 1) * P,
                           j * PAIRW:(j + 1) * PAIRW])

    # startup-critical loads, laid out so each k-group operand (wk[k] +
    # x pair-0[k]) lands just in time for its matmul in the first chunk
    dma_w(nc.scalar, wk_sb[0], wkvT_in[0:P, 0:HIDDEN])
    dma_w(nc.scalar, wk_sb[3], wkvT_in[3 * P:4 * P, 0:HIDDEN])
    dma_x(nc.scalar, 0, 1, 0)          # scalar stops here: exp comes next
    dma_x(nc.sync, 0, 0, 0)
    dma_x(nc.sync, 0, 3, 0)
    dma_x(nc.gpsimd, 0, 2, 0)
    dma_w(nc.gpsimd, wk_sb[2], wkvT_in[2 * P:3 * P, 0:HIDDEN])
    dma_w(nc.gpsimd, wk_sb[1], wkvT_in[P:2 * P, 0:HIDDEN])
    dma_w(nc.gpsimd, wv_sb[1], wkvT_in[P:2 * P, HIDDEN:2 * HIDDEN])
    dma_w(nc.sync, wv_sb[0], wkvT_in[0:P, HIDDEN:2 * HIDDEN])
    dma_w(nc.gpsimd, wv_sb[3], wkvT_in[3 * P:4 * P, HIDDEN:2 * HIDDEN])
    dma_w(nc.sync, wv_sb[2], wkvT_in[2 * P:3 * P, HIDDEN:2 * HIDDEN])
    for j in range(1, NCHUNK // 2):
        for k in range(KT):
            dma_x(nc.sync if (k + j) % 2 == 0 else nc.gpsimd, 0, k, j)

    def load_x_batch1():
        for j in range(NCHUNK // 2):
            for k in range(KT):
                dma_x(nc.sync if (k + j) % 2 == 0 else nc.gpsimd, 1, k, j)

    def x_sb(b, k, i):
        return xj_sb[(b, k, i // 2)][:, (i % 2) * CHUNK:(i % 2 + 1) * CHUNK]

    yrow_sb = {}  # (b, u) -> [128, 4096] fp16
    for b in range(BPC):
        for u in range(MT):
            yrow_sb[(b, u)] = ypool.tile([P, L], F16, tag=f"yrow{b}_{u}",
                                         name=f"yrow{b}_{u}")

    # fold-time weights: tiles declared here, DMAs emitted mid-pass-A (they
    # are not needed until fold, and must not clog any queue at startup)
    wq_sb = [wpool.tile([P, DIM], F16, tag=f"wq{p}", name=f"wq{p}")
             for p in range(NPAIR)]
    woT_sb = [wpool.tile([P, DIM], F16, tag=f"wo{p}", name=f"wo{p}")
              for p in range(NPAIR)]
    bias_sb = wpool.tile([P, MT], F32, tag="bias", name="bias")

    def load_fold_weights():
        for p in range(NPAIR):
            nc.sync.dma_start(wq_sb[p][:], wq_in[p * P:(p + 1) * P, :])
            nc.gpsimd.dma_start(woT_sb[p][:], woT_in[p * P:(p + 1) * P, :])
        nc.gpsimd.dma_start(bias_sb[:], bias_in[:])

    wyT_sb = {}  # (b, m) -> [128 c-sub, 512 y] fp16

    def pass_A(b, y_batch=None, trailing_hook=None, ctx_acc=None):
        """kT/vT projections, exp, ctx accumulation for batch b; optionally
        interleaves the y pass of a previous batch. ctx groups of chunk i-1
        are emitted between the kv/y groups of chunk i so their small-N
        matmuls' LDWEIGHTS hide under big matmuls (queue reorder window).
        trailing_hook(p) is called right after pair p's last ctx group so
        per-pair finalize work overlaps the remaining ctx groups."""
        def ctx_group(i, p, E_t, vT_t):
            pc = psc.tile([P, 2 * VW], F32, tag="ctx", name="ctx")
            for lm in range(LM):
                nc.tensor.matmul(
                    pc[:], E_t[lm][:, p * P:(p + 1) * P],
                    vT_t[lm][:, p * 2 * VW:(p + 1) * 2 * VW],
                    start=(lm == 0), stop=(lm == LM - 1))
            if i == 0:
                nc.vector.tensor_copy(ctx_acc[p][:], pc[:])
            else:
                nc.vector.tensor_add(ctx_acc[p][:], ctx_acc[p][:], pc[:])

        prev = None  # (i-1, E_t, vT_t)
        for i in range(NCHUNK):
            E_t, vT_t = [], []
            # all k-groups first: the exp drains chase them with slack, so
            # the next chunk's kps allocation never stalls on ACT
            for lm in range(LM):
                lsl = slice(lm * P, (lm + 1) * P)
                # during batch 0's pass A the y psum pool is idle — alternate
                # with it so the exp drain never gates the next k-group
                kpool = psy if (y_batch is None and lm % 2 == 1) else psk
                ktag = "yps" if kpool is psy else "kps"
                kps = kpool.tile([P, CHUNK], F32, tag=ktag, name=ktag)
                for k in range(KT):
                    nc.tensor.matmul(kps[:], x_sb(b, k, i)[:, lsl],
                                     wk_sb[k][:],
                                     start=(k == 0), stop=(k == KT - 1))
                e = epool.tile([P, CHUNK], F16, tag="E", name="E")
                nc.scalar.activation(e[:], kps[:],
                                     mybir.ActivationFunctionType.Exp)
                E_t.append(e)
            for lm in range(LM):
                lsl = slice(lm * P, (lm + 1) * P)
                vps = psv.tile([P, CHUNK], F32, tag="vps", name="vps")
                for k in range(KT):
                    nc.tensor.matmul(vps[:], x_sb(b, k, i)[:, lsl],
                                     wv_sb[k][:],
                                     start=(k == 0), stop=(k == KT - 1))
                if prev is not None:
                    ctx_group(prev[0], lm, prev[1], prev[2])
                v = epool.tile([P, HEADS * VW], F16, tag="vT", name="vT")
                v_view = v[:].rearrange("p (h e) -> p h e", e=VW)
                nc.vector.tensor_copy(
                    v_view[:, :, 0:DH],
                    vps[:].rearrange("p (h e) -> p h e", e=DH))
                nc.vector.tensor_scalar(
                    v_view[:, :, DH:DH + 2],
                    vps[:].rearrange("p (h e) -> p h e", e=DH)[:, :, 0:2],
                    0.0, 1.0, mybir.AluOpType.mult, mybir.AluOpType.add)
                vT_t.append(v)
            if y_batch is not None:
                # u-strip halves: (u, chunks 0-3) on even i, (u, 4-8) on odd
                u, half = i // 2, i % 2
                y_ublock(y_batch, u, half * (NCHUNK // 2),
                         (half + 1) * (NCHUNK // 2))
            if b == 0 and i == 1:
                load_x_batch1()
            if b == 0 and i == 3:
                load_fold_weights()
            prev = (i, E_t, vT_t)
        for p in range(NPAIR):
            ctx_group(prev[0], p, prev[1], prev[2])
            if trailing_hook is not None:
                trailing_hook(p)
        return ctx_acc

    tmp_sb = {}

    def fold_pair(b, ctx_acc, p):
        """ctx~ = ctx/rowsum; tmp_h = ctx~_h^T @ wq_h (pair p)."""
        acc = ctx_acc[p]
        nc.vector.reciprocal(acc[0:DH, DH:DH + 1], acc[0:DH, DH:DH + 1])
        nc.vector.reciprocal(acc[DH:P, 2 * VW - 2:2 * VW - 1],
                             acc[DH:P, 2 * VW - 2:2 * VW - 1])
        ctxn = fpool.tile([P, DH], F16, tag=f"ctxn{p}", name=f"ctxn{p}")
        nc.vector.tensor_scalar_mul(ctxn[0:DH, :], acc[0:DH, 0:DH],
                                    acc[0:DH, DH:DH + 1])
        nc.vector.tensor_scalar_mul(ctxn[DH:P, :], acc[DH:P, VW:VW + DH],
                                    acc[DH:P, 2 * VW - 2:2 * VW - 1])
        tps = psy.tile([P, CHUNK], F32, tag="yps", name="yps")
        nc.tensor.matmul(tps[0:DH, :], ctxn[0:DH, :], wq_sb[p][0:DH, :],
                         start=True, stop=True, tile_position=(0, 0))
        nc.tensor.matmul(tps[DH:P, :], ctxn[DH:P, :], wq_sb[p][DH:P, :],
                         start=True, stop=True, tile_position=(DH, DH))
        t = fpool.tile([P, CHUNK], F16, tag=f"tmp{p}", name=f"tmp{p}")
        nc.scalar.copy(t[:], tps[:])
        tmp_sb[p] = t

    def fold_wyT(b):
        """WyT[m] = sum_p tmp-pair contraction with w_out^T."""
        for m in range(MT):
            wps = psy.tile([P, CHUNK], F32, tag="yps", name="yps")
            for p in range(NPAIR):
                nc.tensor.matmul(wps[:], tmp_sb[p][:, m * P:(m + 1) * P],
                                 woT_sb[p][:],
                                 start=(p == 0), stop=(p == NPAIR - 1))
            t = wypool.tile([P, DIM], F16, tag=f"wyt{b}_{m}",
                            name=f"wyt{b}_{m}")
            nc.vector.tensor_copy(t[:], wps[:])
            wyT_sb[(b, m)] = t

    def y_ublock(b, u, i0, i1):
        """y chunks [i0, i1) for output row strip u; DMA when the strip is
        complete so the store overlaps the remaining strips' compute."""
        yrow = yrow_sb[(b, u)]
        for i in range(i0, i1):
            ls = slice(i * CHUNK, (i + 1) * CHUNK)
            yps = psy.tile([P, CHUNK], F32, tag="yps", name="yps")
            for k in range(KT):
                nc.tensor.matmul(yps[:],
                                 wyT_sb[(b, k)][:, u * P:(u + 1) * P],
                                 x_sb(b, k, i),
                                 start=(k == 0), stop=(k == KT - 1))
            # drain on ACT (Identity + per-partition bias): keeps DVE, which
            # carries the vT copies and ctx accumulation, off the critical path
            nc.scalar.activation(yrow[:, ls], yps[:],
                                 mybir.ActivationFunctionType.Identity,
                                 bias=bias_sb[:, u:u + 1])
            # fire the store in pieces as chunks complete so only the final
            # 256KB piece is exposed at the end of the kernel
            if i == 3:
                nc.gpsimd.dma_start(
                    y_out[b, u * P:(u + 1) * P, 0:4 * CHUNK],
                    yrow[:, 0:4 * CHUNK])
            elif i == 6:
                nc.gpsimd.dma_start(
                    y_out[b, u * P:(u + 1) * P, 4 * CHUNK:7 * CHUNK],
                    yrow[:, 4 * CHUNK:7 * CHUNK])
            elif i == 7:
                nc.gpsimd.dma_start(
                    y_out[b, u * P:(u + 1) * P, 7 * CHUNK:L],
                    yrow[:, 7 * CHUNK:L])

    def pass_Y(b):
        """y = WyT^T @ x + bias for batch b; DMA out as fp16."""
        for u in range(MT):
            y_ublock(b, u, 0, NCHUNK)

    acc_box = {}

    def hook0(p):
        fold_pair(0, acc_box[0], p)

    def hook1(p):
        fold_pair(1, acc_box[1], p)

    acc_box[0] = [cpool.tile([P, 2 * VW], F32, tag=f"ctxacc{p}",
                             name=f"ctxacc{p}") for p in range(NPAIR)]
    pass_A(0, y_batch=None, trailing_hook=hook0, ctx_acc=acc_box[0])
    fold_wyT(0)
    acc_box[1] = [cpool.tile([P, 2 * VW], F32, tag=f"ctxacc{p}",
                             name=f"ctxacc{p}") for p in range(NPAIR)]
    pass_A(1, y_batch=0, trailing_hook=hook1, ctx_acc=acc_box[1])
    fold_wyT(1)
    pass_Y(1)


def build_module():
    nc = bacc.Bacc("TRN2", target_bir_lowering=False, debug=False,
                   num_devices=NCORES)
    x_in = nc.dram_tensor("x", [BPC, DIM, L], F16, kind="ExternalInput")
    wkvT_in = nc.dram_tensor("w_kvT", [DIM, 2 * HIDDEN], F16,
                             kind="ExternalInput")
    wq_in = nc.dram_tensor("w_q", [HIDDEN, DIM], F16, kind="ExternalInput")
    woT_in = nc.dram_tensor("w_oT", [HIDDEN, DIM], F16, kind="ExternalInput")
    bias_in = nc.dram_tensor("bias", [P, MT], F32, kind="ExternalInput")
    y_out = nc.dram_tensor("y", [BPC, DIM, L], F16, kind="ExternalOutput")
    with tile.TileContext(nc) as tc:
        with ExitStack() as ctx:
            build_kernel(ctx, tc, x_in, wkvT_in, wq_in, woT_in, bias_in,
                         y_out)
    nc.compile()
    return nc


def make_in_maps(x, w_qkv, w_out, b_out):
    x = np.ascontiguousarray(x, dtype=np.float32).reshape(B, DIM, L)
    x16 = x.astype(np.float16)
    w_qkv = np.asarray(w_qkv, dtype=np.float32)
    wq = np.ascontiguousarray(w_qkv[0:HIDDEN]).astype(np.float16)
    wkvT = np.ascontiguousarray(
        np.concatenate([w_qkv[HIDDEN:2 * HIDDEN].T,
                        w_qkv[2 * HIDDEN:3 * HIDDEN].T], axis=1)
    ).astype(np.float16)
    woT = np.ascontiguousarray(
        np.asarray(w_out, dtype=np.float32).T).astype(np.float16)
    bias = np.ascontiguousarray(
        np.asarray(b_out, dtype=np.float32).reshape(MT, P).T)
    in_maps = []
    for c in range(NCORES):
        in_maps.append({
            "x": x16[c * BPC:(c + 1) * BPC],
            "w_kvT": wkvT,
            "w_q": wq,
            "w_oT": woT,
            "bias": bias,
        })
    return in_maps


_NC_CACHE = None


def kernel(x, w_qkv, w_out, b_out, *, trace=False, trace_kwargs=None):
    """Full inputs in, full output out. Shards batch across 8 NeuronCores."""
    global _NC_CACHE
    from concourse.bass_utils import run_bass_kernel_spmd

    if _NC_CACHE is None:
        _NC_CACHE = build_module()
    nc = _NC_CACHE

    in_maps = make_in_maps(x, w_qkv, w_out, b_out)
    kw = dict(trace_kwargs or {})
    res = run_bass_kernel_spmd(nc, in_maps, list(range(NCORES)),
                               trace=trace, **kw)
    y = np.empty((B, DIM, HGT, WID), dtype=np.float32)
    for c in range(NCORES):
        y[c * BPC:(c + 1) * BPC] = res.results[c]["y"].astype(
            np.float32).reshape(BPC, DIM, HGT, WID)
    kernel.last_results = res
    return y
